# revision 1
# baseline (speedup 1.0000x reference)
"""Trainium2 Bass kernel for the CMIN video encoder (2x banded MHA + BiGRU).

Self-contained: builds one SPMD Bass program, shards batch across the
8 NeuronCores (8 batches each), runs via run_bass_kernel_spmd, and
reassembles the full [64, 256, 512] output on the host.

Layout strategy: activations feature-major ([feature, token]); projections
are lhsT=weightT matmuls. Banded attention computed per (b, h) on the two
128x128 diagonal score blocks plus two 3-wide corner blocks (the band never
leaves them); q/k/v staged in bf16 and loaded once per head. The BiGRU runs
both direction chains interleaved; W_hh in fp8e4 (x64 prescale, un-scaled
inside the gate adds); h lives in fp32 in a per-chunk SBUF y-tile (with an
fp16 shadow as the matmul moving operand). Every 32 steps the y-tile is
PE-transposed to token-major and indirect-DMA-scattered straight into the
output tensor - sequence reversal, placement and tail masking all encoded
in host-built row-index tables, keeping the program SPMD-identical.
"""

import os
import numpy as np
import concourse.bass as bass
import concourse.bacc as bacc
import concourse.tile as tile
import concourse.mybir as mybir
from concourse.bass_utils import run_bass_kernel_spmd

B, T, D = 64, 256, 1024
H, DK = 8, D // 8
HID = 512
GH = HID >> 1          # 256
G3 = 3 * GH            # 768
ATTN_WIDTH = 3
NL = 2
NCORES = 8
BC = B // NCORES       # 8 batches per core
NTOK = BC * T          # 2048 token columns per core
SCALE = 1.0 / float(np.sqrt(DK))

F32 = mybir.dt.float32
F32R = mybir.dt.float32r
F16 = mybir.dt.float16
BF16 = mybir.dt.bfloat16
F8 = mybir.dt.float8e4
I32 = mybir.dt.int32
AF = mybir.ActivationFunctionType
ALU = mybir.AluOpType

KC = D // 128          # 8 contraction chunks for D
GC = G3 // 128         # 6 gate chunks
HC = GH // 128         # 2 hidden chunks
TT = NTOK // 512       # 4 token tiles of 512
TC = T // 128          # 2 chunks of the T axis
CH = 32                # recurrence steps per gx stream chunk
NCHUNK = T // CH

YR = BC * T            # valid yout rows; row YR is the trash row
W8 = True              # W_hh in fp8e4 (prescaled); False -> fp16
WHH_SCALE = 64.0 if W8 else 1.0
WHH_INV = 1.0 / WHH_SCALE
CW = 262               # valid score columns: 2x128 diag + 2x3 corners


def _build(repeat: int = 1, phases: str = 'all'):
    nc = bacc.Bacc("TRN2", num_devices=NCORES)

    xT = nc.dram_tensor("xT", [D, NTOK], F32R, kind="ExternalInput")
    wq, wk, wv, wo = [], [], [], []
    for l in range(NL):
        wq.append(nc.dram_tensor(f"WqT{l}", [D, D], BF16, kind="ExternalInput"))
        wk.append(nc.dram_tensor(f"WkT{l}", [D, D], BF16, kind="ExternalInput"))
        wv.append(nc.dram_tensor(f"WvT{l}", [D, D], BF16, kind="ExternalInput"))
        wo.append(nc.dram_tensor(f"WoT{l}", [D, D], BF16, kind="ExternalInput"))
    wihf = nc.dram_tensor("WihFT", [D, G3], F32R, kind="ExternalInput")
    wihb = nc.dram_tensor("WihBT", [D, G3], BF16, kind="ExternalInput")
    whhf = nc.dram_tensor("WhhFT", [GH, G3], F8 if W8 else F16,
                          kind="ExternalInput")
    whhb = nc.dram_tensor("WhhBT", [GH, G3], F8 if W8 else F16,
                          kind="ExternalInput")
    band_d = nc.dram_tensor("band", [128, 264], BF16, kind="ExternalInput")
    ones_d = nc.dram_tensor("ones", [128, 128], BF16, kind="ExternalInput")
    iden_d = nc.dram_tensor("iden", [128, 128], F32, kind="ExternalInput")
    gxidx_d = nc.dram_tensor("gxidx", [128, NCHUNK * 2], I32, kind="ExternalInput")
    sidx_d = nc.dram_tensor("sidx", [128, NCHUNK * 4], I32, kind="ExternalInput")
    yout = nc.dram_tensor("yout", [YR + 1, HID], F16, kind="ExternalOutput")

    with (
        nc.allow_low_precision(reason="bf16/fp16/fp8 staging is deliberate"),
        tile.TileContext(nc) as tc,
        tc.tile_pool(name="dram", bufs=1, space="DRAM") as dpool,
        tc.tile_pool(name="const", bufs=1) as cpool,
        tc.tile_pool(name="xs", bufs=1) as xpool,
        tc.tile_pool(name="stage", bufs=6) as spool,
    ):
        qf_d = dpool.tile([H, 128, NTOK], BF16, name="qf_d")
        kf_d = dpool.tile([H, 128, NTOK], BF16, name="kf_d")
        vt_d = dpool.tile([BC, H, TC, 128, 128], BF16, name="vt_d")
        gxb_d = dpool.tile([BC * T, G3], F32, name="gxb_d")

        # ---- constants ---------------------------------------------------
        band_t = cpool.tile([128, 264], BF16, name="band_t")
        nc.sync.dma_start(band_t[:], band_d[:])
        ones_t = cpool.tile([128, 128], BF16, name="ones_t")
        nc.sync.dma_start(ones_t[:], ones_d[:])
        iden_t = cpool.tile([128, 128], F32, name="iden_t")
        nc.sync.dma_start(iden_t[:], iden_d[:])
        gxidx_t = cpool.tile([128, NCHUNK * 2], I32, name="gxidx_t")
        nc.sync.dma_start(gxidx_t[:], gxidx_d[:])
        sidx_t = cpool.tile([128, NCHUNK * 4], I32, name="sidx_t")
        nc.sync.dma_start(sidx_t[:], sidx_d[:])
        whh_t = cpool.tile([128, 2 * HC * G3], F8 if W8 else F16, name="whh_t")
        for dr, wd in enumerate((whhf, whhb)):
            nc.sync.dma_start(
                whh_t[:, dr * HC * G3:(dr + 1) * HC * G3]
                .rearrange("p (c g) -> p c g", c=HC),
                wd[:, :].rearrange("(c p) g -> p c g", p=128),
            )
        hzero = cpool.tile([128, 2 * HC * BC], F16, name="hzero")
        nc.vector.memset(hzero[:], 0.0)

        # ---- x resident (feature-major) ---------------------------------
        x_t = xpool.tile([128, KC * NTOK], F32R, name="x_t")
        nc.sync.dma_start(
            x_t[:].rearrange("p (c n) -> p c n", c=KC),
            xT[:, :].rearrange("(c p) n -> p c n", p=128),
        )

        x16 = xpool.tile([128, KC * NTOK], BF16, name="x16")
        nc.vector.tensor_copy(x16[:], x_t[:])

        def xsl(kc, c0=0, n=NTOK):
            return x_t[:, kc * NTOK + c0: kc * NTOK + c0 + n]

        def xsl16(kc, c0=0, n=NTOK):
            return x16[:, kc * NTOK + c0: kc * NTOK + c0 + n]

        def load_w_half(wpool, wdram, h0, hw, dt=F32R):
            wt = wpool.tile([128, KC * 512], dt, name="wt", tag="wt")
            nc.sync.dma_start(
                wt[:, 0:KC * hw].rearrange("p (c w) -> p c w", c=KC),
                wdram[:, h0:h0 + hw].rearrange("(c p) w -> p c w", p=128),
            )
            return wt

        def attn_phase(wpool, aopool, bhpool, psA, psB,
                       do_proj=True, do_inner=True):
            ao_t = aopool.tile([128, H * NTOK], BF16, name="ao_t")
            if not do_inner:
                nc.vector.memset(ao_t[:], 0.0)
            for l in range(NL):
                # ============ PASS A: V, K, Q projections ============
                for half in range(2 if do_proj else 0):
                    wt = load_w_half(wpool, wv[l], half * 512, 512, dt=BF16)
                    for tc_i in range(NTOK // 128):
                        ps = psA.tile([128, 512], F32, name="psv", tag="psa")
                        for kc in range(KC):
                            nc.tensor.matmul(
                                ps[:],
                                xsl16(kc, tc_i * 128, 128),
                                wt[:, kc * 512:(kc + 1) * 512],
                                start=(kc == 0),
                                stop=(kc == KC - 1),
                            )
                        st = spool.tile([128, 512], BF16, name="stv", tag="st")
                        nc.vector.tensor_copy(st[:], ps[:])
                        b_i, c_i = tc_i // TC, tc_i % TC
                        nc.sync.dma_start(
                            vt_d[b_i, half * 4:(half + 1) * 4, c_i, :, :]
                            .rearrange("h p d -> p h d"),
                            st[:].rearrange("p (h d) -> p h d", h=4),
                        )
                for which, (wdram, outd) in enumerate(
                        ((wk[l], kf_d), (wq[l], qf_d)) if do_proj else ()):
                    for half in range(2):
                        wt = load_w_half(wpool, wdram, half * 512, 512, dt=BF16)
                        for mcl in range(4):
                            mc = half * 4 + mcl
                            for tt in range(TT):
                                ps = psA.tile([128, 512], F32, name="psa", tag="psa")
                                for kc in range(KC):
                                    nc.tensor.matmul(
                                        ps[:],
                                        wt[:, kc * 512 + mcl * 128: kc * 512 + (mcl + 1) * 128],
                                        xsl16(kc, tt * 512, 512),
                                        start=(kc == 0),
                                        stop=(kc == KC - 1),
                                    )
                                st = spool.tile([128, 512], BF16, name="st", tag="st")
                                nc.scalar.activation(st[:], ps[:], AF.Copy)
                                nc.sync.dma_start(
                                    outd[mc, :, tt * 512:(tt + 1) * 512], st[:]
                                )

                # ============ PASS B: banded attention per (b, h) ============
                for h in range(H if do_inner else 0):
                    qh = bhpool.tile([128, NTOK], BF16, name="qh", tag="qh")
                    nc.sync.dma_start(qh[:], qf_d[h])
                    kh = bhpool.tile([128, NTOK], BF16, name="kh", tag="kh")
                    nc.sync.dma_start(kh[:], kf_d[h])
                    vh = bhpool.tile([128, NTOK], BF16, name="vh", tag="vh")
                    for c in range(TC):
                        nc.sync.dma_start(
                            vh[:].rearrange("p (b c d) -> p b c d", b=BC, c=TC)[
                                :, :, c, :
                            ],
                            vt_d[:, h, c].rearrange("b p d -> p b d"),
                        )
                    for b0 in range(0, BC, 2):
                        dn = psB.tile([128, 512], F32, name="dn", tag="dn")
                        rr = bhpool.tile([128, 512], BF16, name="rr", tag="rr")
                        avs = []
                        for bl in range(2):
                            b = b0 + bl
                            qb = qh[:, b * T:(b + 1) * T]
                            kb = kh[:, b * T:(b + 1) * T]
                            vb = vh[:, b * T:(b + 1) * T]
                            ps = psB.tile([128, 264], F32, name="psst", tag="psst")
                            nc.tensor.matmul(ps[:, 0:128], kb[:, 0:128],
                                             qb[:, 0:128], start=True, stop=True)
                            nc.tensor.matmul(ps[:, 256:259], kb[:, 0:128],
                                             qb[:, 128:131], start=True, stop=True)
                            nc.tensor.matmul(ps[:, 128:256], kb[:, 128:256],
                                             qb[:, 128:256], start=True, stop=True)
                            nc.tensor.matmul(ps[:, 259:262], kb[:, 128:256],
                                             qb[:, 125:128], start=True, stop=True)
                            pe = bhpool.tile([128, 264], BF16, name="pe", tag="pe")
                            nc.scalar.activation(pe[:, 0:CW], ps[:, 0:CW],
                                                 AF.Exp, scale=SCALE)
                            pm = bhpool.tile([128, 264], BF16, name="pm", tag="pm")
                            nc.vector.tensor_mul(pm[:, 0:CW], pe[:, 0:CW],
                                                 band_t[:, 0:CW])
                            dsl = dn[:, bl * 256:(bl + 1) * 256]
                            nc.tensor.matmul(dsl[:, 0:128], ones_t[:],
                                             pm[:, 0:128], start=True, stop=False)
                            nc.tensor.matmul(dsl[:, 125:128], ones_t[:, 0:128],
                                             pm[:, 259:262], start=False, stop=True,
                                             skip_group_check=True)
                            nc.tensor.matmul(dsl[:, 128:256], ones_t[:],
                                             pm[:, 128:256], start=True, stop=False)
                            nc.tensor.matmul(dsl[:, 128:131], ones_t[:, 0:128],
                                             pm[:, 256:259], start=False, stop=True,
                                             skip_group_check=True)
                            av = psB.tile([128, 256], F32, name="av", tag="av")
                            nc.tensor.matmul(av[:, 0:128], vb[:, 0:128],
                                             pm[:, 0:128], start=True, stop=False)
                            nc.tensor.matmul(av[:, 125:128], vb[:, 128:256],
                                             pm[:, 259:262], start=False, stop=True,
                                             skip_group_check=True)
                            nc.tensor.matmul(av[:, 128:256], vb[:, 128:256],
                                             pm[:, 128:256], start=True, stop=False)
                            nc.tensor.matmul(av[:, 128:131], vb[:, 0:128],
                                             pm[:, 256:259], start=False, stop=True,
                                             skip_group_check=True)
                            avs.append(av)
                        nc.vector.reciprocal(rr[:], dn[:])
                        for bl in range(2):
                            b = b0 + bl
                            nc.vector.tensor_mul(
                                ao_t[:, h * NTOK + b * T: h * NTOK + (b + 1) * T],
                                avs[bl][:], rr[:, bl * 256:(bl + 1) * 256],
                            )

                # ============ PASS C: O projection + residual (in place) =====
                for half in range(2 if do_proj else 0):
                    wt = load_w_half(wpool, wo[l], half * 512, 512, dt=BF16)
                    for mcl in range(4):
                        mc = half * 4 + mcl
                        for tt in range(TT):
                            ps = psA.tile([128, 512], F32, name="pso", tag="psa")
                            for kc in range(KC):
                                nc.tensor.matmul(
                                    ps[:],
                                    wt[:, kc * 512 + mcl * 128: kc * 512 + (mcl + 1) * 128],
                                    ao_t[:, kc * NTOK + tt * 512: kc * NTOK + (tt + 1) * 512],
                                    start=(kc == 0),
                                    stop=(kc == KC - 1),
                                )
                            nc.vector.tensor_add(
                                xsl(mc, tt * 512, 512), ps[:], xsl(mc, tt * 512, 512)
                            )
                            nc.vector.tensor_copy(
                                xsl16(mc, tt * 512, 512), xsl(mc, tt * 512, 512)
                            )

            # ============ PASS D (bwd half): gx_bwd -> DRAM ============
            for half in range(2 if do_proj else 0):
                wt = load_w_half(wpool, wihb, half * 384, 384, dt=BF16)
                for mcl in range(3):
                    mc = half * 3 + mcl
                    for tt in range(TT):
                        ps = psA.tile([128, 512], F32, name="psg", tag="psa")
                        for kc in range(KC):
                            nc.tensor.matmul(
                                ps[:],
                                wt[:, kc * 384 + mcl * 128: kc * 384 + (mcl + 1) * 128],
                                xsl16(kc, tt * 512, 512),
                                start=(kc == 0),
                                stop=(kc == KC - 1),
                            )
                        st = spool.tile([128, 512], F32, name="stg", tag="st")
                        nc.scalar.activation(st[:], ps[:], AF.Copy)
                        nc.sync.dma_start(
                            gxb_d[:, :]
                            .rearrange("(b t) g -> b t g", b=BC)[
                                tt * 2:(tt + 1) * 2, :, mc * 128:(mc + 1) * 128
                            ]
                            .rearrange("b t g -> g (b t)"),
                            st[:],
                        )

        def gru_phase(gxpool, recpool, psR):
            wf_t = gxpool.tile([128, KC * G3], F32R, name="wf_t", tag="wf", bufs=1)
            nc.sync.dma_start(
                wf_t[:].rearrange("p (c g) -> p c g", c=KC),
                wihf[:, :].rearrange("(c p) g -> p c g", p=128),
            )
            h16prev = None
            for ck in range(NCHUNK):
                gxs = gxpool.tile([128, CH * 96], F32, name="gxs", tag="gxs")
                # fwd gx: compute directly into SBUF for this time chunk
                for mc in range(GC):
                    ps = psR.tile([128, 256], F32, name="psf", tag="psf")
                    for kc in range(KC):
                        nc.tensor.matmul(
                            ps[:],
                            wf_t[:, kc * G3 + mc * 128: kc * G3 + (mc + 1) * 128],
                            x_t[:, kc * NTOK:(kc + 1) * NTOK]
                            .rearrange("p (b t) -> p b t", b=BC)[:, :, ck * CH:(ck + 1) * CH],
                            start=(kc == 0),
                            stop=(kc == KC - 1),
                        )
                    nc.vector.tensor_copy(
                        gxs[:, :]
                        .rearrange("p (j d c b) -> p j d c b", j=CH, d=2, c=GC)[
                            :, :, 0, mc, :
                        ].rearrange("p j b -> p b j"),
                        ps[:].rearrange("p (b j) -> p b j", b=BC),
                    )
                # bwd gx: indirect row gather in reverse_padded order + transpose
                for hf2 in range(2):
                    gb = gxpool.tile([128, G3], F32, name="gb", tag="gb", bufs=2)
                    nc.gpsimd.indirect_dma_start(
                        out=gb[:],
                        out_offset=None,
                        in_=gxb_d[:, :],
                        in_offset=bass.IndirectOffsetOnAxis(
                            ap=gxidx_t[:, ck * 2 + hf2: ck * 2 + hf2 + 1], axis=0
                        ),
                    )
                    for c in range(GC):
                        tp = psR.tile([128, 128], F32, name="tp", tag="tp")
                        nc.tensor.transpose(
                            tp[:], gb[:, c * 128:(c + 1) * 128], iden_t[:]
                        )
                        nc.vector.tensor_copy(
                            gxs[:, :]
                            .rearrange("p (j d c b) -> p j d c b", j=CH, d=2, c=GC)[
                                :, :, 1, c, hf2 * 4:(hf2 + 1) * 4
                            ]
                            .rearrange("p j b -> p b j"),
                            tp[:].rearrange("p (b j) -> p b j", b=4),
                        )
                # y/h tile: [128, (j, dr, c, b)] fp16; the matmul moving
                # operand, the h for the gate blend, and the staged y are
                # all this one tile.
                h16t = recpool.tile([128, CH * 32], F16, name="h16t",
                                    tag="h16t", bufs=2)
                for jj in range(CH):
                    gsl = gxs[:, jj * 96:(jj + 1) * 96]
                    if jj == 0:
                        hs16 = hzero if h16prev is None else h16prev
                        hoff = 0 if h16prev is None else (CH - 1) * 32
                    else:
                        hs16, hoff = h16t, (jj - 1) * 32
                    for dr in range(2):
                        ps_g = psR.tile([128, 48], F32, name=f"ps_g{dr}",
                                        tag=f"ps_g{dr}")
                        for c in range(GC):
                            for kc in range(HC):
                                nc.tensor.matmul(
                                    ps_g[:, c * 8:(c + 1) * 8],
                                    whh_t[:, (dr * HC + kc) * G3 + c * 128:
                                          (dr * HC + kc) * G3 + (c + 1) * 128],
                                    hs16[:, hoff + dr * 16 + kc * 8:
                                         hoff + dr * 16 + (kc + 1) * 8],
                                    start=(kc == 0),
                                    stop=(kc == HC - 1),
                                )
                        grz = recpool.tile([128, 32], F32, name="grz", tag=f"grz{dr}")
                        nc.vector.scalar_tensor_tensor(
                            grz[:], ps_g[:, 0:32], WHH_INV,
                            gsl[:, dr * 48: dr * 48 + 32],
                            op0=ALU.mult, op1=ALU.add,
                        )
                        rz = recpool.tile([128, 32], F32, name="rz", tag=f"rz{dr}")
                        nc.scalar.activation(rz[:], grz[:], AF.Sigmoid)
                        t1 = recpool.tile([128, 16], F32, name="t1", tag=f"t1{dr}")
                        nc.vector.tensor_mul(t1[:], rz[:, 0:16], ps_g[:, 32:48])
                        t2 = recpool.tile([128, 16], F32, name="t2", tag=f"t2{dr}")
                        nc.vector.scalar_tensor_tensor(
                            t2[:], t1[:], WHH_INV,
                            gsl[:, dr * 48 + 32: dr * 48 + 48],
                            op0=ALU.mult, op1=ALU.add,
                        )
                        n_t = recpool.tile([128, 16], F32, name="n_t", tag=f"n_t{dr}")
                        nc.scalar.activation(n_t[:], t2[:], AF.Tanh)
                        d_t = recpool.tile([128, 16], F32, name="d_t", tag=f"d_t{dr}")
                        nc.gpsimd.tensor_sub(
                            d_t[:], hs16[:, hoff + dr * 16: hoff + (dr + 1) * 16],
                            n_t[:],
                        )
                        zd = recpool.tile([128, 16], F32, name="zd", tag=f"zd{dr}")
                        nc.gpsimd.tensor_mul(zd[:], rz[:, 16:32], d_t[:])
                        nc.vector.tensor_add(
                            h16t[:, jj * 32 + dr * 16: jj * 32 + (dr + 1) * 16],
                            n_t[:], zd[:],
                        )
                # transpose to token-major and scatter into yout
                for dr in range(2):
                    for jh in range(2):
                        yrp = recpool.tile([128, 256], F32, name="yrp",
                                           tag="yrp", bufs=2)
                        for c in range(HC):
                            nc.vector.tensor_copy(
                                yrp[:, c * 128:(c + 1) * 128]
                                .rearrange("p (j b) -> p j b", j=16),
                                h16t[:, :]
                                .rearrange("p (j d c b) -> p j d c b",
                                           j=CH, d=2, c=HC)[
                                    :, jh * 16:(jh + 1) * 16, dr, c, :
                                ],
                            )
                        tp = psR.tile([128, 256], F32, name="tps", tag="tp")
                        for c in range(HC):
                            nc.tensor.transpose(
                                tp[:, c * 128:(c + 1) * 128],
                                yrp[:, c * 128:(c + 1) * 128],
                                iden_t[:],
                            )
                        yst = recpool.tile([128, 256], F16, name="yst",
                                           tag="yst", bufs=3)
                        nc.vector.tensor_copy(yst[:], tp[:])
                        col = ck * 4 + dr * 2 + jh
                        # sidx holds 2*row so coef=GH lands on row*HID; the
                        # direction's column offset rides in element_offset.
                        nc.gpsimd.indirect_dma_start(
                            out=yout[:, 0:GH],
                            out_offset=bass.IndirectOffsetOnAxis(
                                ap=sidx_t[:, col:col + 1], axis=0
                            ),
                            in_=yst[:],
                            in_offset=None,
                            element_offset=dr * GH,
                        )
                h16prev = h16t

        for rep in range(repeat):
            if phases in ("all", "attn", "proj", "inner"):
                with (
                    tc.tile_pool(name="wt", bufs=2) as wpool,
                    tc.tile_pool(name="ao", bufs=1) as aopool,
                    tc.tile_pool(name="bh", bufs=3) as bhpool,
                    tc.tile_pool(name="psA", bufs=2, space="PSUM") as psA,
                    tc.tile_pool(name="psB", bufs=2, space="PSUM") as psB,
                ):
                    attn_phase(wpool, aopool, bhpool, psA, psB,
                               do_proj=(phases != "inner"),
                               do_inner=(phases != "proj"))
            if phases in ("all", "gru"):
                with (
                    tc.tile_pool(name="gx", bufs=2) as gxpool,
                    tc.tile_pool(name="rec", bufs=3) as recpool,
                    tc.tile_pool(name="psR", bufs=2, space="PSUM") as psR,
                ):
                    gru_phase(gxpool, recpool, psR)

    nc.compile()
    return nc


_NC_CACHE = {}


def _get_nc(repeat: int = 1):
    if repeat not in _NC_CACHE:
        _NC_CACHE[repeat] = _build(repeat)
    return _NC_CACHE[repeat]


def _host_inputs(inputs, core):
    import ml_dtypes
    f8 = ml_dtypes.float8_e4m3

    bs = slice(core * BC, (core + 1) * BC)
    seg = np.asarray(inputs["seg_feats"][bs])
    seglen = np.asarray(inputs["seglen"][bs]).astype(np.int64)

    m = {
        "xT": np.ascontiguousarray(
            seg.transpose(2, 0, 1).reshape(D, NTOK), dtype=np.float32
        )
    }
    for l in range(NL):
        for nm_in, nm_out in (("Wq", "WqT"), ("Wk", "WkT"), ("Wv", "WvT")):
            m[f"{nm_out}{l}"] = np.ascontiguousarray(
                np.asarray(inputs[nm_in][l]).T).astype(ml_dtypes.bfloat16)
        m[f"WoT{l}"] = np.ascontiguousarray(
            np.asarray(inputs["Wo"][l]).T).astype(ml_dtypes.bfloat16)
    m["WihFT"] = np.ascontiguousarray(np.asarray(inputs["W_ih_f"]).T, np.float32)
    m["WihBT"] = np.ascontiguousarray(np.asarray(inputs["W_ih_b"]).T).astype(ml_dtypes.bfloat16)
    # biases are all zero in this model; the kernel skips them entirely
    for l in range(NL):
        for w in "qkvo":
            assert not np.any(np.asarray(inputs[f"b{w}"][l])), \
                "nonzero attention biases unsupported"
    for nm in ("b_ih_f", "b_ih_b", "b_hh_f", "b_hh_b"):
        assert not np.any(np.asarray(inputs[nm])), "nonzero GRU biases unsupported"
    wtype = f8 if W8 else np.float16
    m["WhhFT"] = np.ascontiguousarray(
        np.asarray(inputs["W_hh_f"]).T * WHH_SCALE).astype(wtype)
    m["WhhBT"] = np.ascontiguousarray(
        np.asarray(inputs["W_hh_b"]).T * WHH_SCALE).astype(wtype)

    # band mask: two 128x128 diagonal blocks + two 3-wide corner blocks
    band = np.zeros((128, 264), np.float32)
    p = np.arange(128)
    for c in range(2):
        band[:, c * 128:(c + 1) * 128] = (
            np.abs(p[:, None] - p[None, :]) <= ATTN_WIDTH
        )
    for j in range(3):
        for pp in range(125, 128):           # corner A: k=pp, q=128+j
            if abs(pp - 128 - j) <= ATTN_WIDTH:
                band[pp, 256 + j] = 1.0
        for pp in range(0, 3):               # corner B: k=128+pp, q=125+j
            if abs(128 + pp - 125 - j) <= ATTN_WIDTH:
                band[pp, 259 + j] = 1.0
    m["band"] = band.astype(ml_dtypes.bfloat16)
    m["ones"] = np.ones((128, 128), ml_dtypes.bfloat16)
    m["iden"] = np.eye(128, dtype=np.float32)

    gxidx = np.zeros((128, NCHUNK * 2), np.int32)
    for ck in range(NCHUNK):
        for hf2 in range(2):
            col = ck * 2 + hf2
            for bl in range(4):
                b = hf2 * 4 + bl
                L = int(seglen[b])
                for jl in range(CH):
                    j = ck * CH + jl
                    src_t = min(max(L - 1 - j, 0), T - 1)
                    gxidx[bl * CH + jl, col] = b * T + src_t
    m["gxidx"] = gxidx

    # scatter rows: partition p = (jl, b) of the transposed y block.
    # Values are 2*row: the kernel's scatter AP has coef=GH (=HID/2), so
    # doubling here makes the index land on row*HID.
    sidx = np.full((128, NCHUNK * 4), 2 * YR, np.int32)
    for ck in range(NCHUNK):
        for dr in range(2):
            for jh in range(2):
                col = ck * 4 + dr * 2 + jh
                for jl in range(16):
                    j = ck * CH + jh * 16 + jl
                    for b in range(BC):
                        L = int(seglen[b])
                        if j < L:
                            t = j if dr == 0 else L - 1 - j
                            sidx[jl * 8 + b, col] = 2 * (b * T + t)
    m["sidx"] = sidx
    return m


def kernel(**inputs) -> np.ndarray:
    repeat = int(os.environ.get("KERNEL_REPEAT", "1"))
    nc = _get_nc(repeat)
    in_maps = [_host_inputs(inputs, c) for c in range(NCORES)]
    res = run_bass_kernel_spmd(nc, in_maps, core_ids=list(range(NCORES)))
    out = np.stack([
        res.results[c]["yout"][0:YR].reshape(BC, T, HID) for c in range(NCORES)
    ])
    return np.ascontiguousarray(
        out.reshape(B, T, HID), dtype=np.float32
    )



# revision 12
# speedup vs baseline: 3.2267x; 3.2267x over previous
"""Trainium2 Bass kernel for the CMIN video encoder (2x banded MHA + BiGRU).

Self-contained: builds one SPMD Bass program, shards batch across the
8 NeuronCores (8 batches each), runs via run_bass_kernel_spmd, and
reassembles the full [64, 256, 512] output on the host.

Layout: activations feature-major f16 ([feature, token]); projections are
lhsT=weightT matmuls. Attention is fused per head entirely in SBUF (q/k/v
never touch DRAM); the band never leaves the two 128x128 diagonal score
blocks plus two 3-wide corners. gx_bwd is staged to DRAM token-major via
PE transposes + contiguous DMA (no scatter). The BiGRU runs both direction
chains interleaved; W_hh in fp8e4 (x64 prescale); gx is injected into the
gate PSUM group by a 64*I matmul so the sigmoid reads PSUM directly with
the free affine 1/64 scale. Every 32 steps the y-tile is PE-transposed to
token-major and indirect-DMA-scattered into the output tensor - sequence
reversal, placement and tail masking all encoded in host-built row-index
tables, keeping the program SPMD-identical.
"""

import os
import numpy as np
import concourse.bass as bass
import concourse.bacc as bacc
import concourse.tile as tile
import concourse.mybir as mybir
from concourse.bass_utils import run_bass_kernel_spmd

B, T, D = 64, 256, 1024
H, DK = 8, D // 8
HID = 512
GH = HID >> 1          # 256
G3 = 3 * GH            # 768
ATTN_WIDTH = 3
NL = 2
NCORES = 8
BC = B // NCORES       # 8 batches per core
NTOK = BC * T          # 2048 token columns per core
SCALE = 1.0 / float(np.sqrt(DK))

F32 = mybir.dt.float32
F16 = mybir.dt.float16
F8 = mybir.dt.float8e4
I32 = mybir.dt.int32
AF = mybir.ActivationFunctionType
ALU = mybir.AluOpType

KC = D // 128          # 8 contraction chunks for D
GC = G3 // 128         # 6 gate chunks
HC = GH // 128         # 2 hidden chunks
TT = NTOK // 512       # 4 token tiles of 512
TC = T // 128          # 2 chunks of the T axis
CH = 32                # recurrence steps per gx stream chunk
NCHUNK = T // CH

YR = BC * T            # valid output tokens; row 2*YR is the trash row
WHH_SCALE = 64.0
WHH_INV = 1.0 / WHH_SCALE
CW = 262               # valid score columns: 2x128 diag + 2x3 corners


def _build(repeat: int = 1):
    nc = bacc.Bacc("TRN2", num_devices=NCORES)

    xT = nc.dram_tensor("xT", [D, NTOK], F16, kind="ExternalInput")
    wq, wk, wv, wo = [], [], [], []
    for l in range(NL):
        wq.append(nc.dram_tensor(f"WqT{l}", [D, D], F16, kind="ExternalInput"))
        wk.append(nc.dram_tensor(f"WkT{l}", [D, D], F16, kind="ExternalInput"))
        wv.append(nc.dram_tensor(f"WvT{l}", [D, D], F16, kind="ExternalInput"))
        wo.append(nc.dram_tensor(f"WoT{l}", [D, D], F16, kind="ExternalInput"))
    wihf = nc.dram_tensor("WihFT", [D, G3], F16, kind="ExternalInput")
    wihb = nc.dram_tensor("WihBT", [D, G3], F16, kind="ExternalInput")
    whhf = nc.dram_tensor("WhhFT", [GH, G3], F8, kind="ExternalInput")
    whhb = nc.dram_tensor("WhhBT", [GH, G3], F8, kind="ExternalInput")
    band_d = nc.dram_tensor("band", [128, 264], F16, kind="ExternalInput")
    ones_d = nc.dram_tensor("ones", [128, 128], F16, kind="ExternalInput")
    iden_d = nc.dram_tensor("iden", [128, 128], F16, kind="ExternalInput")
    iden64_d = nc.dram_tensor("iden64", [128, 128], F16, kind="ExternalInput")
    gxidx_d = nc.dram_tensor("gxidx", [128, NCHUNK * 2], I32, kind="ExternalInput")
    sidx_d = nc.dram_tensor("sidx", [128, NCHUNK * 4], I32, kind="ExternalInput")
    yout = nc.dram_tensor("yout", [2 * YR + 2, GH], F16, kind="ExternalOutput")

    with (
        nc.allow_low_precision(reason="f16/fp8 staging is deliberate"),
        tile.TileContext(nc) as tc,
        tc.tile_pool(name="dram", bufs=1, space="DRAM") as dpool,
        tc.tile_pool(name="const", bufs=1) as cpool,
        tc.tile_pool(name="xs", bufs=1) as xpool,
        tc.tile_pool(name="stage", bufs=6) as spool,
    ):
        gxb_d = dpool.tile([NTOK, G3], F16, name="gxb_d")

        # ---- constants ---------------------------------------------------
        band_t = cpool.tile([128, 264], F16, name="band_t")
        nc.sync.dma_start(band_t[:], band_d[:])
        ones_t = cpool.tile([128, 128], F16, name="ones_t")
        nc.sync.dma_start(ones_t[:], ones_d[:])
        iden_t = cpool.tile([128, 128], F16, name="iden_t")
        nc.sync.dma_start(iden_t[:], iden_d[:])
        iden64_t = cpool.tile([128, 128], F16, name="iden64_t")
        nc.sync.dma_start(iden64_t[:], iden64_d[:])
        gxidx_t = cpool.tile([128, NCHUNK * 2], I32, name="gxidx_t")
        nc.sync.dma_start(gxidx_t[:], gxidx_d[:])
        sidx_t = cpool.tile([128, NCHUNK * 4], I32, name="sidx_t")
        nc.sync.dma_start(sidx_t[:], sidx_d[:])
        whh_t = cpool.tile([128, 2 * HC * G3], F8, name="whh_t")
        for dr, wd in enumerate((whhf, whhb)):
            nc.sync.dma_start(
                whh_t[:, dr * HC * G3:(dr + 1) * HC * G3]
                .rearrange("p (c g) -> p c g", c=HC),
                wd[:, :].rearrange("(c p) g -> p c g", p=128),
            )
        hzero = cpool.tile([128, 2 * HC * BC], F16, name="hzero")
        nc.vector.memset(hzero[:], 0.0)

        # ---- x resident (feature-major, f16) -----------------------------
        x_t = xpool.tile([128, KC * NTOK], F16, name="x_t")
        nc.sync.dma_start(
            x_t[:].rearrange("p (c n) -> p c n", c=KC),
            xT[:, :].rearrange("(c p) n -> p c n", p=128),
        )

        def xsl(kc, c0=0, n=NTOK):
            return x_t[:, kc * NTOK + c0: kc * NTOK + c0 + n]

        def attn_phase(wpool, aopool, bhpool, psA, psB):
            ao_t = aopool.tile([128, H * NTOK], F16, name="ao_t")
            for l in range(NL):
                # full-weight loads for q/k/v (sliced per head below)
                wq_t = wpool.tile([128, KC * D], F16, name="wq_t", tag="wq",
                                  bufs=1)
                wk_t = wpool.tile([128, KC * D], F16, name="wk_t", tag="wk",
                                  bufs=1)
                wv_t = wpool.tile([128, KC * D], F16, name="wv_t", tag="wv",
                                  bufs=1)
                for wt_, wd_ in ((wq_t, wq[l]), (wk_t, wk[l]), (wv_t, wv[l])):
                    nc.sync.dma_start(
                        wt_[:].rearrange("p (c d) -> p c d", c=KC),
                        wd_[:, :].rearrange("(c p) d -> p c d", p=128),
                    )
                # ---- V projection, token-major (the av matmuls contract
                # over k-tokens on partitions): vt[tok, (blk, h, dk)] ----
                vt = aopool.tile([128, (NTOK // 128) * H * 128], F16,
                                 name="vt_t", tag="vt")
                for half in range(2):
                    for blk in range(NTOK // 128):
                        ps = psA.tile([128, 512], F32, name="psv", tag="psa")
                        for kc in range(KC):
                            nc.tensor.matmul(
                                ps[:],
                                xsl(kc, blk * 128, 128),
                                wv_t[:, kc * D + half * 512:
                                     kc * D + half * 512 + 512],
                                start=(kc == 0),
                                stop=(kc == KC - 1),
                            )
                        nc.vector.tensor_copy(
                            vt[:].rearrange("p (blk h d) -> p blk h d",
                                            blk=NTOK // 128, h=H)[
                                :, blk, half * 4:(half + 1) * 4, :
                            ],
                            ps[:].rearrange("p (h d) -> p h d", h=4),
                        )
                for h in range(H):
                    # ---- per-head q/k projection (SBUF only) ----
                    qh = bhpool.tile([128, NTOK], F16, name="qh", tag="qh")
                    kh = bhpool.tile([128, NTOK], F16, name="kh", tag="kh")
                    for wt_, outd, eng in (
                        (wk_t, kh, nc.scalar),
                        (wq_t, qh, nc.vector),
                    ):
                        for tt in range(TT):
                            ps = psA.tile([128, 512], F32, name="psp", tag="psa")
                            for kc in range(KC):
                                nc.tensor.matmul(
                                    ps[:],
                                    wt_[:, kc * D + h * 128: kc * D + (h + 1) * 128],
                                    xsl(kc, tt * 512, 512),
                                    start=(kc == 0),
                                    stop=(kc == KC - 1),
                                )
                            if eng is nc.scalar:
                                nc.scalar.activation(
                                    outd[:, tt * 512:(tt + 1) * 512], ps[:],
                                    AF.Copy)
                            else:
                                nc.vector.tensor_copy(
                                    outd[:, tt * 512:(tt + 1) * 512], ps[:])

                    # ---- banded attention for this head ----
                    for b0 in range(0, BC, 2):
                        dn = psB.tile([128, 512], F32, name="dn", tag="dn")
                        rr = bhpool.tile([128, 512], F16, name="rr", tag="rr")
                        avs = []
                        for bl in range(2):
                            b = b0 + bl
                            qb = qh[:, b * T:(b + 1) * T]
                            kb = kh[:, b * T:(b + 1) * T]
                            vb0 = vt[:, ((b * TC + 0) * H + h) * 128:
                                     ((b * TC + 0) * H + h) * 128 + 128]
                            vb1 = vt[:, ((b * TC + 1) * H + h) * 128:
                                     ((b * TC + 1) * H + h) * 128 + 128]
                            ps = psB.tile([128, 264], F32, name="psst", tag="psst")
                            nc.tensor.matmul(ps[:, 0:128], kb[:, 0:128],
                                             qb[:, 0:128], start=True, stop=True)
                            nc.tensor.matmul(ps[:, 256:259], kb[:, 0:128],
                                             qb[:, 128:131], start=True, stop=True)
                            nc.tensor.matmul(ps[:, 128:256], kb[:, 128:256],
                                             qb[:, 128:256], start=True, stop=True)
                            nc.tensor.matmul(ps[:, 259:262], kb[:, 128:256],
                                             qb[:, 125:128], start=True, stop=True)
                            pe = bhpool.tile([128, 264], F16, name="pe", tag="pe")
                            nc.scalar.activation(pe[:, 0:CW], ps[:, 0:CW],
                                                 AF.Exp, scale=SCALE)
                            pm = bhpool.tile([128, 264], F16, name="pm", tag="pm")
                            nc.vector.tensor_mul(pm[:, 0:CW], pe[:, 0:CW],
                                                 band_t[:, 0:CW])
                            dsl = dn[:, bl * 256:(bl + 1) * 256]
                            nc.tensor.matmul(dsl[:, 0:128], ones_t[:],
                                             pm[:, 0:128], start=True, stop=False)
                            nc.tensor.matmul(dsl[:, 125:128], ones_t[:, 0:128],
                                             pm[:, 259:262], start=False, stop=True,
                                             skip_group_check=True)
                            nc.tensor.matmul(dsl[:, 128:256], ones_t[:],
                                             pm[:, 128:256], start=True, stop=False)
                            nc.tensor.matmul(dsl[:, 128:131], ones_t[:, 0:128],
                                             pm[:, 256:259], start=False, stop=True,
                                             skip_group_check=True)
                            av = psB.tile([128, 256], F32, name="av", tag="av")
                            nc.tensor.matmul(av[:, 0:128], vb0,
                                             pm[:, 0:128], start=True, stop=False)
                            nc.tensor.matmul(av[:, 125:128], vb1,
                                             pm[:, 259:262], start=False, stop=True,
                                             skip_group_check=True)
                            nc.tensor.matmul(av[:, 128:256], vb1,
                                             pm[:, 128:256], start=True, stop=False)
                            nc.tensor.matmul(av[:, 128:131], vb0,
                                             pm[:, 256:259], start=False, stop=True,
                                             skip_group_check=True)
                            avs.append(av)
                        nc.vector.reciprocal(rr[:], dn[:])
                        for bl in range(2):
                            b = b0 + bl
                            nc.vector.tensor_mul(
                                ao_t[:, h * NTOK + b * T: h * NTOK + (b + 1) * T],
                                avs[bl][:], rr[:, bl * 256:(bl + 1) * 256],
                            )

                # ---- O projection + residual (in place) ----
                for half in range(2):
                    wo_t = wpool.tile([128, KC * 512], F16, name="wo_t",
                                      tag="wo", bufs=1)
                    nc.sync.dma_start(
                        wo_t[:].rearrange("p (c w) -> p c w", c=KC),
                        wo[l][:, half * 512:(half + 1) * 512]
                        .rearrange("(c p) w -> p c w", p=128),
                    )
                    for mcl in range(4):
                        mc = half * 4 + mcl
                        for tt in range(TT):
                            ps = psA.tile([128, 512], F32, name="pso", tag="psa")
                            for kc in range(KC):
                                nc.tensor.matmul(
                                    ps[:],
                                    wo_t[:, kc * 512 + mcl * 128: kc * 512 + (mcl + 1) * 128],
                                    ao_t[:, kc * NTOK + tt * 512: kc * NTOK + (tt + 1) * 512],
                                    start=(kc == 0),
                                    stop=(kc == KC - 1),
                                )
                            nc.vector.tensor_add(
                                xsl(mc, tt * 512, 512), ps[:], xsl(mc, tt * 512, 512)
                            )

        def gxb_phase(wpool, psA, psB):
            # gx_bwd -> DRAM, token-major via PE transposes (contiguous DMA)
            wb_t = wpool.tile([128, KC * G3], F16, name="wb_t", tag="wb",
                              bufs=1)
            nc.sync.dma_start(
                wb_t[:].rearrange("p (c g) -> p c g", c=KC),
                wihb[:, :].rearrange("(c p) g -> p c g", p=128),
            )
            for tt in range(TT):
                stg = wpool.tile([128, 4 * G3], F16, name="stg", tag="stg",
                                 bufs=2)
                for mc in range(GC):
                    ps = psA.tile([128, 512], F32, name="psg", tag="psa")
                    for kc in range(KC):
                        nc.tensor.matmul(
                            ps[:],
                            wb_t[:, kc * G3 + mc * 128: kc * G3 + (mc + 1) * 128],
                            xsl(kc, tt * 512, 512),
                            start=(kc == 0),
                            stop=(kc == KC - 1),
                        )
                    st = spool.tile([128, 512], F16, name="stg16", tag="st")
                    nc.scalar.activation(st[:], ps[:], AF.Copy)
                    for sub in range(4):
                        tp = psB.tile([128, 128], F16, name="tpd", tag="psst")
                        nc.tensor.transpose(
                            tp[:], st[:, sub * 128:(sub + 1) * 128], iden_t[:]
                        )
                        nc.vector.tensor_copy(
                            stg[:, sub * G3 + mc * 128: sub * G3 + (mc + 1) * 128],
                            tp[:],
                        )
                nc.sync.dma_start(
                    gxb_d[tt * 512:(tt + 1) * 512, :]
                    .rearrange("(sub p) g -> p sub g", p=128),
                    stg[:].rearrange("p (sub g) -> p sub g", sub=4),
                )

        def gru_phase(gxpool, recpool, psR):
            wf_t = gxpool.tile([128, KC * G3], F16, name="wf_t", tag="wf", bufs=1)
            nc.sync.dma_start(
                wf_t[:].rearrange("p (c g) -> p c g", c=KC),
                wihf[:, :].rearrange("(c p) g -> p c g", p=128),
            )
            h16prev = None
            for ck in range(NCHUNK):
                gxs = gxpool.tile([128, CH * 96], F16, name="gxs", tag="gxs")
                # fwd gx: compute directly into SBUF for this time chunk
                for mc in range(GC):
                    ps = psR.tile([128, 256], F32, name="psf", tag="psf")
                    for kc in range(KC):
                        nc.tensor.matmul(
                            ps[:],
                            wf_t[:, kc * G3 + mc * 128: kc * G3 + (mc + 1) * 128],
                            x_t[:, kc * NTOK:(kc + 1) * NTOK]
                            .rearrange("p (b t) -> p b t", b=BC)[:, :, ck * CH:(ck + 1) * CH],
                            start=(kc == 0),
                            stop=(kc == KC - 1),
                        )
                    nc.vector.tensor_copy(
                        gxs[:, :]
                        .rearrange("p (j d c b) -> p j d c b", j=CH, d=2, c=GC)[
                            :, :, 0, mc, :
                        ].rearrange("p j b -> p b j"),
                        ps[:].rearrange("p (b j) -> p b j", b=BC),
                    )
                # bwd gx: indirect row gather in reverse_padded order + transpose
                for hf2 in range(2):
                    gb = gxpool.tile([128, G3], F16, name="gb", tag="gb", bufs=2)
                    nc.gpsimd.indirect_dma_start(
                        out=gb[:],
                        out_offset=None,
                        in_=gxb_d[:, :],
                        in_offset=bass.IndirectOffsetOnAxis(
                            ap=gxidx_t[:, ck * 2 + hf2: ck * 2 + hf2 + 1], axis=0
                        ),
                    )
                    for c in range(GC):
                        tp = psR.tile([128, 128], F16, name="tp", tag="tp")
                        nc.tensor.transpose(
                            tp[:], gb[:, c * 128:(c + 1) * 128], iden_t[:]
                        )
                        nc.vector.tensor_copy(
                            gxs[:, :]
                            .rearrange("p (j d c b) -> p j d c b", j=CH, d=2, c=GC)[
                                :, :, 1, c, hf2 * 4:(hf2 + 1) * 4
                            ]
                            .rearrange("p j b -> p b j"),
                            tp[:].rearrange("p (b j) -> p b j", b=4),
                        )
                # y/h tile: [128, (j, dr, c, b)] fp16; the matmul moving
                # operand, the h for the gate blend, and the staged y are
                # all this one tile.
                h16t = recpool.tile([128, CH * 32], F16, name="h16t",
                                    tag="h16t", bufs=2)
                for jj in range(CH):
                    gsl = gxs[:, jj * 96:(jj + 1) * 96]
                    if jj == 0:
                        hs16 = hzero if h16prev is None else h16prev
                        hoff = 0 if h16prev is None else (CH - 1) * 32
                    else:
                        hs16, hoff = h16t, (jj - 1) * 32
                    for dr in range(2):
                        ps_g = psR.tile([128, 48], F32, name=f"ps_g{dr}",
                                        tag=f"ps_g{dr}")
                        # inject 64*gx for the r,z gates; whh mms accumulate
                        nc.tensor.matmul(
                            ps_g[:, 0:32], iden64_t[:],
                            gsl[:, dr * 48: dr * 48 + 32],
                            start=True, stop=False,
                        )
                        # r,z gates: accumulate onto the inject (cols 0:32)
                        for c in range(4):
                            for kc in range(HC):
                                nc.tensor.matmul(
                                    ps_g[:, c * 8:(c + 1) * 8],
                                    whh_t[:, (dr * HC + kc) * G3 + c * 128:
                                          (dr * HC + kc) * G3 + (c + 1) * 128],
                                    hs16[:, hoff + dr * 16 + kc * 8:
                                         hoff + dr * 16 + (kc + 1) * 8],
                                    start=False,
                                    stop=(c == 3 and kc == HC - 1),
                                    skip_group_check=True,
                                )
                        # n gate: own group (cols 32:48)
                        for c in range(4, GC):
                            for kc in range(HC):
                                nc.tensor.matmul(
                                    ps_g[:, c * 8:(c + 1) * 8],
                                    whh_t[:, (dr * HC + kc) * G3 + c * 128:
                                          (dr * HC + kc) * G3 + (c + 1) * 128],
                                    hs16[:, hoff + dr * 16 + kc * 8:
                                         hoff + dr * 16 + (kc + 1) * 8],
                                    start=(kc == 0),
                                    stop=(kc == HC - 1),
                                )
                        rz = recpool.tile([128, 32], F32, name="rz", tag=f"rz{dr}")
                        nc.scalar.activation(rz[:], ps_g[:, 0:32], AF.Sigmoid,
                                             scale=WHH_INV)
                        # off the critical chain: zc = 1 - z, zh = z * h
                        zc = recpool.tile([128, 16], F32, name="zc", tag=f"zc{dr}")
                        nc.vector.scalar_tensor_tensor(
                            zc[:], rz[:, 16:32], -1.0, ones_t[:, 0:16],
                            op0=ALU.mult, op1=ALU.add,
                        )
                        zh = recpool.tile([128, 16], F32, name="zh", tag=f"zh{dr}")
                        nc.gpsimd.tensor_mul(
                            zh[:], rz[:, 16:32],
                            hs16[:, hoff + dr * 16: hoff + (dr + 1) * 16],
                        )
                        t1 = recpool.tile([128, 16], F32, name="t1", tag=f"t1{dr}")
                        nc.vector.tensor_mul(t1[:], rz[:, 0:16], ps_g[:, 32:48])
                        t2 = recpool.tile([128, 16], F32, name="t2", tag=f"t2{dr}")
                        nc.vector.scalar_tensor_tensor(
                            t2[:], t1[:], WHH_INV,
                            gsl[:, dr * 48 + 32: dr * 48 + 48],
                            op0=ALU.mult, op1=ALU.add,
                        )
                        n_t = recpool.tile([128, 16], F32, name="n_t", tag=f"n_t{dr}")
                        nc.scalar.activation(n_t[:], t2[:], AF.Tanh)
                        u_t = recpool.tile([128, 16], F32, name="u_t", tag=f"u_t{dr}")
                        nc.vector.tensor_mul(u_t[:], zc[:], n_t[:])
                        nc.vector.tensor_add(
                            h16t[:, jj * 32 + dr * 16: jj * 32 + (dr + 1) * 16],
                            u_t[:], zh[:],
                        )
                # transpose to token-major and scatter into yout
                for dr in range(2):
                    for jh in range(2):
                        yrp = recpool.tile([128, 256], F16, name="yrp",
                                           tag="yrp", bufs=2)
                        for c in range(HC):
                            nc.vector.tensor_copy(
                                yrp[:, c * 128:(c + 1) * 128]
                                .rearrange("p (j b) -> p j b", j=16),
                                h16t[:, :]
                                .rearrange("p (j d c b) -> p j d c b",
                                           j=CH, d=2, c=HC)[
                                    :, jh * 16:(jh + 1) * 16, dr, c, :
                                ],
                            )
                        tp = psR.tile([128, 256], F16, name="tps", tag="tp")
                        for c in range(HC):
                            nc.tensor.transpose(
                                tp[:, c * 128:(c + 1) * 128],
                                yrp[:, c * 128:(c + 1) * 128],
                                iden_t[:],
                            )
                        yst = recpool.tile([128, 256], F16, name="yst",
                                           tag="yst", bufs=3)
                        nc.vector.tensor_copy(yst[:], tp[:])
                        col = ck * 4 + dr * 2 + jh
                        # sidx rows hold 2*(b*T+t)+dr: yout is [2*YR+2, GH]
                        # so that lands on token row (b*T+t), direction half dr.
                        nc.gpsimd.indirect_dma_start(
                            out=yout[:, :],
                            out_offset=bass.IndirectOffsetOnAxis(
                                ap=sidx_t[:, col:col + 1], axis=0
                            ),
                            in_=yst[:],
                            in_offset=None,
                        )
                h16prev = h16t

        for rep in range(repeat):
            with (
                tc.tile_pool(name="wt", bufs=1) as wpool,
                tc.tile_pool(name="ao", bufs=1) as aopool,
                tc.tile_pool(name="bh", bufs=2) as bhpool,
                tc.tile_pool(name="psA", bufs=2, space="PSUM") as psA,
                tc.tile_pool(name="psB", bufs=2, space="PSUM") as psB,
            ):
                attn_phase(wpool, aopool, bhpool, psA, psB)
            with (
                tc.tile_pool(name="wt2", bufs=1) as wpool2,
                tc.tile_pool(name="psA2", bufs=2, space="PSUM") as psA2,
                tc.tile_pool(name="psB2", bufs=2, space="PSUM") as psB2,
            ):
                gxb_phase(wpool2, psA2, psB2)
            with (
                tc.tile_pool(name="gx", bufs=2) as gxpool,
                tc.tile_pool(name="rec", bufs=3) as recpool,
                tc.tile_pool(name="psR", bufs=2, space="PSUM") as psR,
            ):
                gru_phase(gxpool, recpool, psR)

    nc.compile()
    return nc


_NC_CACHE = {}


def _get_nc(repeat: int = 1):
    if repeat not in _NC_CACHE:
        _NC_CACHE[repeat] = _build(repeat)
    return _NC_CACHE[repeat]


def _host_inputs(inputs, core):
    import ml_dtypes
    f8 = ml_dtypes.float8_e4m3

    bs = slice(core * BC, (core + 1) * BC)
    seg = np.asarray(inputs["seg_feats"][bs])
    seglen = np.asarray(inputs["seglen"][bs]).astype(np.int64)

    m = {
        "xT": np.ascontiguousarray(
            seg.transpose(2, 0, 1).reshape(D, NTOK)
        ).astype(np.float16)
    }
    for l in range(NL):
        for nm_in, nm_out in (("Wq", "WqT"), ("Wk", "WkT"), ("Wv", "WvT"),
                              ("Wo", "WoT")):
            m[f"{nm_out}{l}"] = np.ascontiguousarray(
                np.asarray(inputs[nm_in][l]).T).astype(np.float16)
    m["WihFT"] = np.ascontiguousarray(
        np.asarray(inputs["W_ih_f"]).T).astype(np.float16)
    m["WihBT"] = np.ascontiguousarray(
        np.asarray(inputs["W_ih_b"]).T).astype(np.float16)
    # biases are all zero in this model; the kernel skips them entirely
    for l in range(NL):
        for w in "qkvo":
            assert not np.any(np.asarray(inputs[f"b{w}"][l])), \
                "nonzero attention biases unsupported"
    for nm in ("b_ih_f", "b_ih_b", "b_hh_f", "b_hh_b"):
        assert not np.any(np.asarray(inputs[nm])), "nonzero GRU biases unsupported"
    m["WhhFT"] = np.ascontiguousarray(
        np.asarray(inputs["W_hh_f"]).T * WHH_SCALE).astype(f8)
    m["WhhBT"] = np.ascontiguousarray(
        np.asarray(inputs["W_hh_b"]).T * WHH_SCALE).astype(f8)

    # band mask: two 128x128 diagonal blocks + two 3-wide corner blocks
    band = np.zeros((128, 264), np.float32)
    p = np.arange(128)
    for c in range(2):
        band[:, c * 128:(c + 1) * 128] = (
            np.abs(p[:, None] - p[None, :]) <= ATTN_WIDTH
        )
    for j in range(3):
        for pp in range(125, 128):           # corner A: k=pp, q=128+j
            if abs(pp - 128 - j) <= ATTN_WIDTH:
                band[pp, 256 + j] = 1.0
        for pp in range(0, 3):               # corner B: k=128+pp, q=125+j
            if abs(128 + pp - 125 - j) <= ATTN_WIDTH:
                band[pp, 259 + j] = 1.0
    m["band"] = band.astype(np.float16)
    m["ones"] = np.ones((128, 128), np.float16)
    m["iden"] = np.eye(128, dtype=np.float16)
    m["iden64"] = (WHH_SCALE * np.eye(128)).astype(np.float16)

    gxidx = np.zeros((128, NCHUNK * 2), np.int32)
    for ck in range(NCHUNK):
        for hf2 in range(2):
            col = ck * 2 + hf2
            for bl in range(4):
                b = hf2 * 4 + bl
                L = int(seglen[b])
                for jl in range(CH):
                    j = ck * CH + jl
                    src_t = min(max(L - 1 - j, 0), T - 1)
                    gxidx[bl * CH + jl, col] = b * T + src_t
    m["gxidx"] = gxidx

    # scatter rows: partition p = (jl, b) of the transposed y block.
    # yout is [2*YR+2, GH]; row 2*(b*T+t)+dr is token (b,t), direction dr.
    sidx = np.full((128, NCHUNK * 4), 2 * YR, np.int32)
    for ck in range(NCHUNK):
        for dr in range(2):
            for jh in range(2):
                col = ck * 4 + dr * 2 + jh
                for jl in range(16):
                    j = ck * CH + jh * 16 + jl
                    for b in range(BC):
                        L = int(seglen[b])
                        if j < L:
                            t = j if dr == 0 else L - 1 - j
                            sidx[jl * 8 + b, col] = 2 * (b * T + t) + dr
    m["sidx"] = sidx
    return m


def core_output(yout_arr):
    return np.asarray(yout_arr)[0:2 * YR].reshape(BC, T, HID)


def kernel(**inputs) -> np.ndarray:
    repeat = int(os.environ.get("KERNEL_REPEAT", "1"))
    nc = _get_nc(repeat)
    in_maps = [_host_inputs(inputs, c) for c in range(NCORES)]
    res = run_bass_kernel_spmd(nc, in_maps, core_ids=list(range(NCORES)))
    out = np.stack([core_output(res.results[c]["yout"]) for c in range(NCORES)])
    return np.ascontiguousarray(
        out.reshape(B, T, HID), dtype=np.float32
    )


# revision 17
# speedup vs baseline: 4.5297x; 1.4038x over previous
"""Trainium2 Bass kernel for the CMIN video encoder (2x banded MHA + BiGRU).

Self-contained: builds one SPMD Bass program, shards batch across the
8 NeuronCores (8 batches each), runs via run_bass_kernel_spmd, and
reassembles the full [64, 256, 512] output on the host.

Layout: activations feature-major f16 ([feature, token]); projections are
lhsT=weightT matmuls. Attention is fused per head entirely in SBUF (q/k/v
never touch DRAM); the band never leaves the two 128x128 diagonal score
blocks plus two 3-wide corners. gx_bwd is staged to DRAM token-major via
PE transposes + contiguous DMA (no scatter). The BiGRU runs both direction
chains interleaved; W_hh in fp8e4 (x64 prescale); gx is injected into the
gate PSUM group by a 64*I matmul so the sigmoid reads PSUM directly with
the free affine 1/64 scale. Every 32 steps the y-tile is PE-transposed to
token-major and indirect-DMA-scattered into the output tensor - sequence
reversal, placement and tail masking all encoded in host-built row-index
tables, keeping the program SPMD-identical.
"""

import os
import numpy as np
import concourse.bass as bass
import concourse.bacc as bacc
import concourse.tile as tile
import concourse.mybir as mybir
from concourse.bass_utils import run_bass_kernel_spmd

B, T, D = 64, 256, 1024
H, DK = 8, D // 8
HID = 512
GH = HID >> 1          # 256
G3 = 3 * GH            # 768
ATTN_WIDTH = 3
NL = 2
NCORES = 8
BC = B // NCORES       # 8 batches per core
NTOK = BC * T          # 2048 token columns per core
SCALE = 1.0 / float(np.sqrt(DK))

F32 = mybir.dt.float32
F16 = mybir.dt.float16
F8 = mybir.dt.float8e4
I32 = mybir.dt.int32
AF = mybir.ActivationFunctionType
ALU = mybir.AluOpType

KC = D // 128          # 8 contraction chunks for D
GC = G3 // 128         # 6 gate chunks
HC = GH // 128         # 2 hidden chunks
TT = NTOK // 512       # 4 token tiles of 512
TC = T // 128          # 2 chunks of the T axis
CH = 32                # recurrence steps per gx stream chunk
NCHUNK = T // CH

YR = BC * T            # valid output tokens; row 2*YR is the trash row
WHH_SCALE = 64.0
WHH_INV = 1.0 / WHH_SCALE
CW = 262               # valid score columns: 2x128 diag + 2x3 corners


def _build(repeat: int = 1, phases: str = "all"):
    nc = bacc.Bacc("TRN2", num_devices=NCORES)

    xT = nc.dram_tensor("xT", [D, NTOK], F16, kind="ExternalInput")
    wq, wk, wv, wo = [], [], [], []
    for l in range(NL):
        wq.append(nc.dram_tensor(f"WqT{l}", [D, D], F16, kind="ExternalInput"))
        wk.append(nc.dram_tensor(f"WkT{l}", [D, D], F16, kind="ExternalInput"))
        wv.append(nc.dram_tensor(f"WvT{l}", [D, D], F16, kind="ExternalInput"))
        wo.append(nc.dram_tensor(f"WoT{l}", [D, D], F16, kind="ExternalInput"))
    wihf = nc.dram_tensor("WihFT", [D, G3], F16, kind="ExternalInput")
    wihb = nc.dram_tensor("WihBT", [D, G3], F16, kind="ExternalInput")
    whhf = nc.dram_tensor("WhhFT", [GH, G3], F8, kind="ExternalInput")
    whhb = nc.dram_tensor("WhhBT", [GH, G3], F8, kind="ExternalInput")
    band_d = nc.dram_tensor("band", [128, 264], F16, kind="ExternalInput")
    ones_d = nc.dram_tensor("ones", [128, 128], F16, kind="ExternalInput")
    iden_d = nc.dram_tensor("iden", [128, 128], F16, kind="ExternalInput")
    iden64_d = nc.dram_tensor("iden64", [128, 128], F16, kind="ExternalInput")
    gxidx_d = nc.dram_tensor("gxidx", [128, NCHUNK * 2], I32, kind="ExternalInput")
    sidx_d = nc.dram_tensor("sidx", [128, NCHUNK * 4], I32, kind="ExternalInput")
    yout = nc.dram_tensor("yout", [2 * YR + 2, GH], F16, kind="ExternalOutput")

    with (
        nc.allow_low_precision(reason="f16/fp8 staging is deliberate"),
        tile.TileContext(nc) as tc,
        tc.tile_pool(name="dram", bufs=1, space="DRAM") as dpool,
        tc.tile_pool(name="const", bufs=1) as cpool,
        tc.tile_pool(name="xs", bufs=1) as xpool,
        tc.tile_pool(name="stage", bufs=6) as spool,
    ):
        gxb_d = dpool.tile([NTOK, G3], F16, name="gxb_d")

        # ---- constants ---------------------------------------------------
        band_t = cpool.tile([128, 264], F16, name="band_t")
        nc.sync.dma_start(band_t[:], band_d[:])
        ones_t = cpool.tile([128, 128], F16, name="ones_t")
        nc.sync.dma_start(ones_t[:], ones_d[:])
        iden_t = cpool.tile([128, 128], F16, name="iden_t")
        nc.sync.dma_start(iden_t[:], iden_d[:])
        iden64_t = cpool.tile([128, 128], F16, name="iden64_t")
        nc.sync.dma_start(iden64_t[:], iden64_d[:])
        gxidx_t = cpool.tile([128, NCHUNK * 2], I32, name="gxidx_t")
        nc.sync.dma_start(gxidx_t[:], gxidx_d[:])
        sidx_t = cpool.tile([128, NCHUNK * 4], I32, name="sidx_t")
        nc.sync.dma_start(sidx_t[:], sidx_d[:])
        whh_t = cpool.tile([128, 2 * HC * G3], F8, name="whh_t")
        for dr, wd in enumerate((whhf, whhb)):
            nc.sync.dma_start(
                whh_t[:, dr * HC * G3:(dr + 1) * HC * G3]
                .rearrange("p (c g) -> p c g", c=HC),
                wd[:, :].rearrange("(c p) g -> p c g", p=128),
            )
        hzero = cpool.tile([128, 2 * HC * BC], F16, name="hzero")
        nc.vector.memset(hzero[:], 0.0)

        # ---- x resident (feature-major, f16) -----------------------------
        x_t = xpool.tile([128, KC * NTOK], F16, name="x_t")
        nc.sync.dma_start(
            x_t[:].rearrange("p (c n) -> p c n", c=KC),
            xT[:, :].rearrange("(c p) n -> p c n", p=128),
        )

        def xsl(kc, c0=0, n=NTOK):
            return x_t[:, kc * NTOK + c0: kc * NTOK + c0 + n]

        def attn_phase(wpool, aopool, bhpool, psA, psB):
            ao_t = aopool.tile([128, H * NTOK], F16, name="ao_t")
            for l in range(NL):
                # full-weight loads for q/k/v (sliced per head below)
                wq_t = wpool.tile([128, KC * D], F16, name="wq_t", tag="wq",
                                  bufs=1)
                wk_t = wpool.tile([128, KC * D], F16, name="wk_t", tag="wk",
                                  bufs=1)
                wv_t = wpool.tile([128, KC * D], F16, name="wv_t", tag="wv",
                                  bufs=1)
                for wt_, wd_ in ((wq_t, wq[l]), (wk_t, wk[l]), (wv_t, wv[l])):
                    nc.sync.dma_start(
                        wt_[:].rearrange("p (c d) -> p c d", c=KC),
                        wd_[:, :].rearrange("(c p) d -> p c d", p=128),
                    )
                # ---- V projection, token-major (the av matmuls contract
                # over k-tokens on partitions): vt[tok, (blk, h, dk)] ----
                vt = aopool.tile([128, (NTOK // 128) * H * 128], F16,
                                 name="vt_t", tag="vt")
                for half in range(2):
                    for blk in range(NTOK // 128):
                        ps = psA.tile([128, 512], F32, name="psv", tag="psa")
                        for kc in range(KC):
                            nc.tensor.matmul(
                                ps[:],
                                xsl(kc, blk * 128, 128),
                                wv_t[:, kc * D + half * 512:
                                     kc * D + half * 512 + 512],
                                start=(kc == 0),
                                stop=(kc == KC - 1),
                            )
                        nc.vector.tensor_copy(
                            vt[:].rearrange("p (blk h d) -> p blk h d",
                                            blk=NTOK // 128, h=H)[
                                :, blk, half * 4:(half + 1) * 4, :
                            ],
                            ps[:].rearrange("p (h d) -> p h d", h=4),
                        )
                for h in range(H):
                    # ---- per-head q/k projection (SBUF only) ----
                    qh = bhpool.tile([128, NTOK], F16, name="qh", tag="qh")
                    kh = bhpool.tile([128, NTOK], F16, name="kh", tag="kh")
                    for wt_, outd, eng in (
                        (wk_t, kh, nc.scalar),
                        (wq_t, qh, nc.vector),
                    ):
                        for tt in range(TT):
                            ps = psA.tile([128, 512], F32, name="psp", tag="psa")
                            for kc in range(KC):
                                nc.tensor.matmul(
                                    ps[:],
                                    wt_[:, kc * D + h * 128: kc * D + (h + 1) * 128],
                                    xsl(kc, tt * 512, 512),
                                    start=(kc == 0),
                                    stop=(kc == KC - 1),
                                )
                            if eng is nc.scalar:
                                nc.scalar.activation(
                                    outd[:, tt * 512:(tt + 1) * 512], ps[:],
                                    AF.Copy)
                            else:
                                nc.vector.tensor_copy(
                                    outd[:, tt * 512:(tt + 1) * 512], ps[:])

                    # ---- banded attention for this head ----
                    for b0 in range(0, BC, 2):
                        dn = psB.tile([128, 512], F32, name="dn", tag="dn")
                        rr = bhpool.tile([128, 512], F16, name="rr", tag="rr")
                        avs = []
                        for bl in range(2):
                            b = b0 + bl
                            qb = qh[:, b * T:(b + 1) * T]
                            kb = kh[:, b * T:(b + 1) * T]
                            vb0 = vt[:, ((b * TC + 0) * H + h) * 128:
                                     ((b * TC + 0) * H + h) * 128 + 128]
                            vb1 = vt[:, ((b * TC + 1) * H + h) * 128:
                                     ((b * TC + 1) * H + h) * 128 + 128]
                            ps = psB.tile([128, 264], F32, name="psst", tag="psst")
                            nc.tensor.matmul(ps[:, 0:128], kb[:, 0:128],
                                             qb[:, 0:128], start=True, stop=True)
                            nc.tensor.matmul(ps[:, 256:259], kb[:, 0:128],
                                             qb[:, 128:131], start=True, stop=True)
                            nc.tensor.matmul(ps[:, 128:256], kb[:, 128:256],
                                             qb[:, 128:256], start=True, stop=True)
                            nc.tensor.matmul(ps[:, 259:262], kb[:, 128:256],
                                             qb[:, 125:128], start=True, stop=True)
                            pe = bhpool.tile([128, 264], F16, name="pe", tag="pe")
                            nc.scalar.activation(pe[:, 0:CW], ps[:, 0:CW],
                                                 AF.Exp, scale=SCALE)
                            pm = bhpool.tile([128, 264], F16, name="pm", tag="pm")
                            nc.vector.tensor_mul(pm[:, 0:CW], pe[:, 0:CW],
                                                 band_t[:, 0:CW])
                            dsl = dn[:, bl * 256:(bl + 1) * 256]
                            nc.tensor.matmul(dsl[:, 0:128], ones_t[:],
                                             pm[:, 0:128], start=True, stop=False)
                            nc.tensor.matmul(dsl[:, 125:128], ones_t[:, 0:128],
                                             pm[:, 259:262], start=False, stop=True,
                                             skip_group_check=True)
                            nc.tensor.matmul(dsl[:, 128:256], ones_t[:],
                                             pm[:, 128:256], start=True, stop=False)
                            nc.tensor.matmul(dsl[:, 128:131], ones_t[:, 0:128],
                                             pm[:, 256:259], start=False, stop=True,
                                             skip_group_check=True)
                            av = psB.tile([128, 256], F32, name="av", tag="av")
                            nc.tensor.matmul(av[:, 0:128], vb0,
                                             pm[:, 0:128], start=True, stop=False)
                            nc.tensor.matmul(av[:, 125:128], vb1,
                                             pm[:, 259:262], start=False, stop=True,
                                             skip_group_check=True)
                            nc.tensor.matmul(av[:, 128:256], vb1,
                                             pm[:, 128:256], start=True, stop=False)
                            nc.tensor.matmul(av[:, 128:131], vb0,
                                             pm[:, 256:259], start=False, stop=True,
                                             skip_group_check=True)
                            avs.append(av)
                        nc.vector.reciprocal(rr[:], dn[:])
                        for bl in range(2):
                            b = b0 + bl
                            nc.vector.tensor_mul(
                                ao_t[:, h * NTOK + b * T: h * NTOK + (b + 1) * T],
                                avs[bl][:], rr[:, bl * 256:(bl + 1) * 256],
                            )

                # ---- O projection + residual (in place) ----
                for half in range(2):
                    wo_t = wpool.tile([128, KC * 512], F16, name="wo_t",
                                      tag="wo", bufs=1)
                    nc.sync.dma_start(
                        wo_t[:].rearrange("p (c w) -> p c w", c=KC),
                        wo[l][:, half * 512:(half + 1) * 512]
                        .rearrange("(c p) w -> p c w", p=128),
                    )
                    for mcl in range(4):
                        mc = half * 4 + mcl
                        for tt in range(TT):
                            ps = psA.tile([128, 512], F32, name="pso", tag="psa")
                            for kc in range(KC):
                                nc.tensor.matmul(
                                    ps[:],
                                    wo_t[:, kc * 512 + mcl * 128: kc * 512 + (mcl + 1) * 128],
                                    ao_t[:, kc * NTOK + tt * 512: kc * NTOK + (tt + 1) * 512],
                                    start=(kc == 0),
                                    stop=(kc == KC - 1),
                                )
                            nc.vector.tensor_add(
                                xsl(mc, tt * 512, 512), ps[:], xsl(mc, tt * 512, 512)
                            )

        def gxb_phase(wpool, psA, psB):
            # gx_bwd -> DRAM, token-major via PE transposes (contiguous DMA)
            wb_t = wpool.tile([128, KC * G3], F16, name="wb_t", tag="wb",
                              bufs=1)
            nc.sync.dma_start(
                wb_t[:].rearrange("p (c g) -> p c g", c=KC),
                wihb[:, :].rearrange("(c p) g -> p c g", p=128),
            )
            for tt in range(TT):
                stg = wpool.tile([128, 4 * G3], F16, name="stg", tag="stg",
                                 bufs=2)
                for mc in range(GC):
                    ps = psA.tile([128, 512], F32, name="psg", tag="psa")
                    for kc in range(KC):
                        nc.tensor.matmul(
                            ps[:],
                            wb_t[:, kc * G3 + mc * 128: kc * G3 + (mc + 1) * 128],
                            xsl(kc, tt * 512, 512),
                            start=(kc == 0),
                            stop=(kc == KC - 1),
                        )
                    st = spool.tile([128, 512], F16, name="stg16", tag="st")
                    nc.scalar.activation(st[:], ps[:], AF.Copy)
                    for sub in range(4):
                        tp = psB.tile([128, 128], F16, name="tpd", tag="psst")
                        nc.tensor.transpose(
                            tp[:], st[:, sub * 128:(sub + 1) * 128], iden_t[:]
                        )
                        nc.vector.tensor_copy(
                            stg[:, sub * G3 + mc * 128: sub * G3 + (mc + 1) * 128],
                            tp[:],
                        )
                nc.sync.dma_start(
                    gxb_d[tt * 512:(tt + 1) * 512, :]
                    .rearrange("(sub p) g -> p sub g", p=128),
                    stg[:].rearrange("p (sub g) -> p sub g", sub=4),
                )

        def gru_phase(gxpool, recpool, psR):
            wf_t = gxpool.tile([128, KC * G3], F16, name="wf_t", tag="wf", bufs=1)
            nc.sync.dma_start(
                wf_t[:].rearrange("p (c g) -> p c g", c=KC),
                wihf[:, :].rearrange("(c p) g -> p c g", p=128),
            )
            h16prev = None
            for ck in range(NCHUNK):
                gxs = gxpool.tile([128, CH * 96], F16, name="gxs", tag="gxs")
                # fwd gx: compute directly into SBUF for this time chunk
                # gxs per-step layout: [xr0 xr1 xz0 xz1 xn0 xn1] (16 each) so
                # both direction chains run as single wide ops.
                for mc in range(GC):
                    ps = psR.tile([128, 256], F32, name="psf", tag="psf")
                    for kc in range(KC):
                        nc.tensor.matmul(
                            ps[:],
                            wf_t[:, kc * G3 + mc * 128: kc * G3 + (mc + 1) * 128],
                            x_t[:, kc * NTOK:(kc + 1) * NTOK]
                            .rearrange("p (b t) -> p b t", b=BC)[:, :, ck * CH:(ck + 1) * CH],
                            start=(kc == 0),
                            stop=(kc == KC - 1),
                        )
                    nc.vector.tensor_copy(
                        gxs[:, :]
                        .rearrange("p (j gp d c2 b) -> p j gp d c2 b",
                                   j=CH, gp=3, d=2, c2=2)[
                            :, :, mc // 2, 0, mc % 2, :
                        ].rearrange("p j b -> p b j"),
                        ps[:].rearrange("p (b j) -> p b j", b=BC),
                    )
                # bwd gx: indirect row gather in reverse_padded order + transpose
                for hf2 in range(2):
                    gb = gxpool.tile([128, G3], F16, name="gb", tag="gb", bufs=2)
                    nc.gpsimd.indirect_dma_start(
                        out=gb[:],
                        out_offset=None,
                        in_=gxb_d[:, :],
                        in_offset=bass.IndirectOffsetOnAxis(
                            ap=gxidx_t[:, ck * 2 + hf2: ck * 2 + hf2 + 1], axis=0
                        ),
                    )
                    for c in range(GC):
                        tp = psR.tile([128, 128], F16, name="tp", tag="tp")
                        nc.tensor.transpose(
                            tp[:], gb[:, c * 128:(c + 1) * 128], iden_t[:]
                        )
                        nc.vector.tensor_copy(
                            gxs[:, :]
                            .rearrange("p (j gp d c2 b) -> p j gp d c2 b",
                                       j=CH, gp=3, d=2, c2=2)[
                                :, :, c // 2, 1, c % 2, hf2 * 4:(hf2 + 1) * 4
                            ]
                            .rearrange("p j b -> p b j"),
                            tp[:].rearrange("p (b j) -> p b j", b=4),
                        )
                # y/h tile: [128, (j, dr, c, b)] fp16; the matmul moving
                # operand, the h for the gate blend, and the staged y are
                # all this one tile.
                h16t = recpool.tile([128, CH * 32], F16, name="h16t",
                                    tag="h16t", bufs=2)
                for jj in range(CH):
                    gsl = gxs[:, jj * 96:(jj + 1) * 96]
                    if jj == 0:
                        hs16 = hzero if h16prev is None else h16prev
                        hoff = 0 if h16prev is None else (CH - 1) * 32
                    else:
                        hs16, hoff = h16t, (jj - 1) * 32
                    hsl = hs16[:, hoff:hoff + 32]
                    ps_g = psR.tile([128, 96], F32, name="ps_g", tag="ps_g")
                    # inject 64*gx for r,z of both dirs; whh mms accumulate.
                    # ps_g cols: [r0 r1 z0 z1 n0 n1] (16 each)
                    nc.tensor.matmul(
                        ps_g[:, 0:64], iden64_t[:], gsl[:, 0:64],
                        start=True, stop=False,
                    )
                    for dr in range(2):
                        for c in range(GC):
                            col = (c // 2) * 32 + dr * 16 + (c % 2) * 8
                            for kc in range(HC):
                                nc.tensor.matmul(
                                    ps_g[:, col:col + 8],
                                    whh_t[:, (dr * HC + kc) * G3 + c * 128:
                                          (dr * HC + kc) * G3 + (c + 1) * 128],
                                    hs16[:, hoff + dr * 16 + kc * 8:
                                         hoff + dr * 16 + (kc + 1) * 8],
                                    start=(c >= 4 and kc == 0),
                                    stop=(dr == 1 and c == 3 and kc == HC - 1)
                                    if c < 4 else (kc == HC - 1),
                                    skip_group_check=True,
                                )
                    rz = recpool.tile([128, 64], F32, name="rz", tag="rz")
                    nc.scalar.activation(rz[:], ps_g[:, 0:64], AF.Sigmoid,
                                         scale=WHH_INV)
                    # off the critical chain: zc = 1 - z, zh = z * h
                    zc = recpool.tile([128, 32], F32, name="zc", tag="zc")
                    nc.gpsimd.tensor_sub(zc[:], ones_t[:, 0:32], rz[:, 32:64])
                    zh = recpool.tile([128, 32], F32, name="zh", tag="zh")
                    nc.gpsimd.tensor_mul(zh[:], rz[:, 32:64], hsl)
                    t1 = recpool.tile([128, 32], F32, name="t1", tag="t1")
                    nc.vector.tensor_mul(t1[:], rz[:, 0:32], ps_g[:, 64:96])
                    t2 = recpool.tile([128, 32], F32, name="t2", tag="t2")
                    nc.vector.scalar_tensor_tensor(
                        t2[:], t1[:], WHH_INV, gsl[:, 64:96],
                        op0=ALU.mult, op1=ALU.add,
                    )
                    n_t = recpool.tile([128, 32], F32, name="n_t", tag="n_t")
                    nc.scalar.activation(n_t[:], t2[:], AF.Tanh)
                    u_t = recpool.tile([128, 32], F32, name="u_t", tag="u_t")
                    nc.vector.tensor_mul(u_t[:], zc[:], n_t[:])
                    nc.vector.tensor_add(
                        h16t[:, jj * 32:(jj + 1) * 32], u_t[:], zh[:],
                    )
                # transpose to token-major and scatter into yout
                for dr in range(2):
                    for jh in range(2):
                        yrp = recpool.tile([128, 256], F16, name="yrp",
                                           tag="yrp", bufs=2)
                        for c in range(HC):
                            nc.vector.tensor_copy(
                                yrp[:, c * 128:(c + 1) * 128]
                                .rearrange("p (j b) -> p j b", j=16),
                                h16t[:, :]
                                .rearrange("p (j d c b) -> p j d c b",
                                           j=CH, d=2, c=HC)[
                                    :, jh * 16:(jh + 1) * 16, dr, c, :
                                ],
                            )
                        tp = psR.tile([128, 256], F16, name="tps", tag="tp")
                        for c in range(HC):
                            nc.tensor.transpose(
                                tp[:, c * 128:(c + 1) * 128],
                                yrp[:, c * 128:(c + 1) * 128],
                                iden_t[:],
                            )
                        yst = recpool.tile([128, 256], F16, name="yst",
                                           tag="yst", bufs=3)
                        nc.vector.tensor_copy(yst[:], tp[:])
                        col = ck * 4 + dr * 2 + jh
                        # sidx rows hold 2*(b*T+t)+dr: yout is [2*YR+2, GH]
                        # so that lands on token row (b*T+t), direction half dr.
                        nc.gpsimd.indirect_dma_start(
                            out=yout[:, :],
                            out_offset=bass.IndirectOffsetOnAxis(
                                ap=sidx_t[:, col:col + 1], axis=0
                            ),
                            in_=yst[:],
                            in_offset=None,
                        )
                h16prev = h16t

        for rep in range(repeat):
            if phases in ("all", "attn"):
                with (
                    tc.tile_pool(name="wt", bufs=1) as wpool,
                    tc.tile_pool(name="ao", bufs=1) as aopool,
                    tc.tile_pool(name="bh", bufs=2) as bhpool,
                    tc.tile_pool(name="psA", bufs=2, space="PSUM") as psA,
                    tc.tile_pool(name="psB", bufs=2, space="PSUM") as psB,
                ):
                    attn_phase(wpool, aopool, bhpool, psA, psB)
            if phases in ("all", "attn", "gxb"):
                with (
                    tc.tile_pool(name="wt2", bufs=1) as wpool2,
                    tc.tile_pool(name="psA2", bufs=2, space="PSUM") as psA2,
                    tc.tile_pool(name="psB2", bufs=2, space="PSUM") as psB2,
                ):
                    gxb_phase(wpool2, psA2, psB2)
            if phases in ("all", "gru"):
                with (
                    tc.tile_pool(name="gx", bufs=2) as gxpool,
                    tc.tile_pool(name="rec", bufs=3) as recpool,
                    tc.tile_pool(name="psR", bufs=2, space="PSUM") as psR,
                ):
                    gru_phase(gxpool, recpool, psR)

    nc.compile()
    return nc


_NC_CACHE = {}


def _get_nc(repeat: int = 1):
    if repeat not in _NC_CACHE:
        _NC_CACHE[repeat] = _build(repeat)
    return _NC_CACHE[repeat]


def _host_inputs(inputs, core):
    import ml_dtypes
    f8 = ml_dtypes.float8_e4m3

    bs = slice(core * BC, (core + 1) * BC)
    seg = np.asarray(inputs["seg_feats"][bs])
    seglen = np.asarray(inputs["seglen"][bs]).astype(np.int64)

    m = {
        "xT": np.ascontiguousarray(
            seg.transpose(2, 0, 1).reshape(D, NTOK)
        ).astype(np.float16)
    }
    for l in range(NL):
        for nm_in, nm_out in (("Wq", "WqT"), ("Wk", "WkT"), ("Wv", "WvT"),
                              ("Wo", "WoT")):
            m[f"{nm_out}{l}"] = np.ascontiguousarray(
                np.asarray(inputs[nm_in][l]).T).astype(np.float16)
    m["WihFT"] = np.ascontiguousarray(
        np.asarray(inputs["W_ih_f"]).T).astype(np.float16)
    m["WihBT"] = np.ascontiguousarray(
        np.asarray(inputs["W_ih_b"]).T).astype(np.float16)
    # biases are all zero in this model; the kernel skips them entirely
    for l in range(NL):
        for w in "qkvo":
            assert not np.any(np.asarray(inputs[f"b{w}"][l])), \
                "nonzero attention biases unsupported"
    for nm in ("b_ih_f", "b_ih_b", "b_hh_f", "b_hh_b"):
        assert not np.any(np.asarray(inputs[nm])), "nonzero GRU biases unsupported"
    m["WhhFT"] = np.ascontiguousarray(
        np.asarray(inputs["W_hh_f"]).T * WHH_SCALE).astype(f8)
    m["WhhBT"] = np.ascontiguousarray(
        np.asarray(inputs["W_hh_b"]).T * WHH_SCALE).astype(f8)

    # band mask: two 128x128 diagonal blocks + two 3-wide corner blocks
    band = np.zeros((128, 264), np.float32)
    p = np.arange(128)
    for c in range(2):
        band[:, c * 128:(c + 1) * 128] = (
            np.abs(p[:, None] - p[None, :]) <= ATTN_WIDTH
        )
    for j in range(3):
        for pp in range(125, 128):           # corner A: k=pp, q=128+j
            if abs(pp - 128 - j) <= ATTN_WIDTH:
                band[pp, 256 + j] = 1.0
        for pp in range(0, 3):               # corner B: k=128+pp, q=125+j
            if abs(128 + pp - 125 - j) <= ATTN_WIDTH:
                band[pp, 259 + j] = 1.0
    m["band"] = band.astype(np.float16)
    m["ones"] = np.ones((128, 128), np.float16)
    m["iden"] = np.eye(128, dtype=np.float16)
    m["iden64"] = (WHH_SCALE * np.eye(128)).astype(np.float16)

    gxidx = np.zeros((128, NCHUNK * 2), np.int32)
    for ck in range(NCHUNK):
        for hf2 in range(2):
            col = ck * 2 + hf2
            for bl in range(4):
                b = hf2 * 4 + bl
                L = int(seglen[b])
                for jl in range(CH):
                    j = ck * CH + jl
                    src_t = min(max(L - 1 - j, 0), T - 1)
                    gxidx[bl * CH + jl, col] = b * T + src_t
    m["gxidx"] = gxidx

    # scatter rows: partition p = (jl, b) of the transposed y block.
    # yout is [2*YR+2, GH]; row 2*(b*T+t)+dr is token (b,t), direction dr.
    sidx = np.full((128, NCHUNK * 4), 2 * YR, np.int32)
    for ck in range(NCHUNK):
        for dr in range(2):
            for jh in range(2):
                col = ck * 4 + dr * 2 + jh
                for jl in range(16):
                    j = ck * CH + jh * 16 + jl
                    for b in range(BC):
                        L = int(seglen[b])
                        if j < L:
                            t = j if dr == 0 else L - 1 - j
                            sidx[jl * 8 + b, col] = 2 * (b * T + t) + dr
    m["sidx"] = sidx
    return m


def core_output(yout_arr):
    return np.asarray(yout_arr)[0:2 * YR].reshape(BC, T, HID)


def kernel(**inputs) -> np.ndarray:
    repeat = int(os.environ.get("KERNEL_REPEAT", "1"))
    nc = _get_nc(repeat)
    in_maps = [_host_inputs(inputs, c) for c in range(NCORES)]
    res = run_bass_kernel_spmd(nc, in_maps, core_ids=list(range(NCORES)))
    out = np.stack([core_output(res.results[c]["yout"]) for c in range(NCORES)])
    return np.ascontiguousarray(
        out.reshape(B, T, HID), dtype=np.float32
    )


# revision 18
# speedup vs baseline: 4.8618x; 1.0733x over previous
"""Trainium2 Bass kernel for the CMIN video encoder (2x banded MHA + BiGRU).

Self-contained: builds one SPMD Bass program, shards batch across the
8 NeuronCores (8 batches each), runs via run_bass_kernel_spmd, and
reassembles the full [64, 256, 512] output on the host.

Layout: activations feature-major f16 ([feature, token]); projections are
lhsT=weightT matmuls. Attention is fused per head entirely in SBUF (q/k/v
never touch DRAM); the band never leaves the two 128x128 diagonal score
blocks plus two 3-wide corners. gx_bwd is staged to DRAM token-major via
PE transposes + contiguous DMA (no scatter). The BiGRU runs both direction
chains interleaved; W_hh in fp8e4 (x64 prescale); gx is injected into the
gate PSUM group by a 64*I matmul so the sigmoid reads PSUM directly with
the free affine 1/64 scale. Every 32 steps the y-tile is PE-transposed to
token-major and indirect-DMA-scattered into the output tensor - sequence
reversal, placement and tail masking all encoded in host-built row-index
tables, keeping the program SPMD-identical.
"""

import os
import numpy as np
import concourse.bass as bass
import concourse.bacc as bacc
import concourse.tile as tile
import concourse.mybir as mybir
from concourse.bass_utils import run_bass_kernel_spmd

B, T, D = 64, 256, 1024
H, DK = 8, D // 8
HID = 512
GH = HID >> 1          # 256
G3 = 3 * GH            # 768
ATTN_WIDTH = 3
NL = 2
NCORES = 8
BC = B // NCORES       # 8 batches per core
NTOK = BC * T          # 2048 token columns per core
SCALE = 1.0 / float(np.sqrt(DK))

F32 = mybir.dt.float32
F16 = mybir.dt.float16
F8 = mybir.dt.float8e4
I32 = mybir.dt.int32
AF = mybir.ActivationFunctionType
ALU = mybir.AluOpType

KC = D // 128          # 8 contraction chunks for D
GC = G3 // 128         # 6 gate chunks
HC = GH // 128         # 2 hidden chunks
TT = NTOK // 512       # 4 token tiles of 512
TC = T // 128          # 2 chunks of the T axis
CH = 32                # recurrence steps per gx stream chunk
NCHUNK = T // CH

YR = BC * T            # valid output tokens; row 2*YR is the trash row
WHH_SCALE = 64.0
WHH_INV = 1.0 / WHH_SCALE
CW = 262               # valid score columns: 2x128 diag + 2x3 corners


def _build(repeat: int = 1, phases: str = "all"):
    nc = bacc.Bacc("TRN2", num_devices=NCORES)

    xT = nc.dram_tensor("xT", [D, NTOK], F16, kind="ExternalInput")
    wq, wk, wv, wo = [], [], [], []
    for l in range(NL):
        wq.append(nc.dram_tensor(f"WqT{l}", [D, D], F16, kind="ExternalInput"))
        wk.append(nc.dram_tensor(f"WkT{l}", [D, D], F16, kind="ExternalInput"))
        wv.append(nc.dram_tensor(f"WvT{l}", [D, D], F16, kind="ExternalInput"))
        wo.append(nc.dram_tensor(f"WoT{l}", [D, D], F16, kind="ExternalInput"))
    wihf = nc.dram_tensor("WihFT", [D, G3], F16, kind="ExternalInput")
    wihb = nc.dram_tensor("WihBT", [D, G3], F16, kind="ExternalInput")
    whhf = nc.dram_tensor("WhhFT", [GH, G3], F8, kind="ExternalInput")
    whhb = nc.dram_tensor("WhhBT", [GH, G3], F8, kind="ExternalInput")
    band_d = nc.dram_tensor("band", [128, 264], F16, kind="ExternalInput")
    ones_d = nc.dram_tensor("ones", [128, 128], F16, kind="ExternalInput")
    iden_d = nc.dram_tensor("iden", [128, 128], F16, kind="ExternalInput")
    iden64_d = nc.dram_tensor("iden64", [128, 128], F16, kind="ExternalInput")
    gxidx_d = nc.dram_tensor("gxidx", [128, NCHUNK * 2], I32, kind="ExternalInput")
    sidx_d = nc.dram_tensor("sidx", [128, NCHUNK * 4], I32, kind="ExternalInput")
    yout = nc.dram_tensor("yout", [2 * YR + 2, GH], F16, kind="ExternalOutput")

    with (
        nc.allow_low_precision(reason="f16/fp8 staging is deliberate"),
        tile.TileContext(nc) as tc,
        tc.tile_pool(name="dram", bufs=1, space="DRAM") as dpool,
        tc.tile_pool(name="const", bufs=1) as cpool,
        tc.tile_pool(name="xs", bufs=1) as xpool,
        tc.tile_pool(name="stage", bufs=6) as spool,
    ):
        gxb_d = dpool.tile([NTOK, G3], F16, name="gxb_d")

        # ---- constants ---------------------------------------------------
        band_t = cpool.tile([128, 264], F16, name="band_t")
        nc.sync.dma_start(band_t[:], band_d[:])
        ones_t = cpool.tile([128, 128], F16, name="ones_t")
        nc.sync.dma_start(ones_t[:], ones_d[:])
        iden_t = cpool.tile([128, 128], F16, name="iden_t")
        nc.sync.dma_start(iden_t[:], iden_d[:])
        iden64_t = cpool.tile([128, 128], F16, name="iden64_t")
        nc.sync.dma_start(iden64_t[:], iden64_d[:])
        gxidx_t = cpool.tile([128, NCHUNK * 2], I32, name="gxidx_t")
        nc.sync.dma_start(gxidx_t[:], gxidx_d[:])
        sidx_t = cpool.tile([128, NCHUNK * 4], I32, name="sidx_t")
        nc.sync.dma_start(sidx_t[:], sidx_d[:])
        whh_t = cpool.tile([128, 2 * HC * G3], F8, name="whh_t")
        for dr, wd in enumerate((whhf, whhb)):
            nc.sync.dma_start(
                whh_t[:, dr * HC * G3:(dr + 1) * HC * G3]
                .rearrange("p (c g) -> p c g", c=HC),
                wd[:, :].rearrange("(c p) g -> p c g", p=128),
            )
        hzero = cpool.tile([128, 2 * HC * BC], F16, name="hzero")
        nc.vector.memset(hzero[:], 0.0)

        # ---- x resident (feature-major, f16) -----------------------------
        x_t = xpool.tile([128, KC * NTOK], F16, name="x_t")
        nc.sync.dma_start(
            x_t[:].rearrange("p (c n) -> p c n", c=KC),
            xT[:, :].rearrange("(c p) n -> p c n", p=128),
        )

        def xsl(kc, c0=0, n=NTOK):
            return x_t[:, kc * NTOK + c0: kc * NTOK + c0 + n]

        def attn_phase(wpool, aopool, bhpool, psA, psB):
            ao_t = aopool.tile([128, H * NTOK], F16, name="ao_t")
            for l in range(NL):
                # full-weight loads for q/k/v (sliced per head below)
                wq_t = wpool.tile([128, KC * D], F16, name="wq_t", tag="wq",
                                  bufs=1)
                wk_t = wpool.tile([128, KC * D], F16, name="wk_t", tag="wk",
                                  bufs=1)
                wv_t = wpool.tile([128, KC * D], F16, name="wv_t", tag="wv",
                                  bufs=1)
                for wt_, wd_ in ((wq_t, wq[l]), (wk_t, wk[l]), (wv_t, wv[l])):
                    nc.sync.dma_start(
                        wt_[:].rearrange("p (c d) -> p c d", c=KC),
                        wd_[:, :].rearrange("(c p) d -> p c d", p=128),
                    )
                # ---- V projection, token-major (the av matmuls contract
                # over k-tokens on partitions): vt[tok, (blk, h, dk)] ----
                vt = aopool.tile([128, (NTOK // 128) * H * 128], F16,
                                 name="vt_t", tag="vt")
                for half in range(2):
                    for blk in range(NTOK // 128):
                        ps = psA.tile([128, 512], F32, name="psv", tag="psa")
                        for kc in range(KC):
                            nc.tensor.matmul(
                                ps[:],
                                xsl(kc, blk * 128, 128),
                                wv_t[:, kc * D + half * 512:
                                     kc * D + half * 512 + 512],
                                start=(kc == 0),
                                stop=(kc == KC - 1),
                            )
                        nc.vector.tensor_copy(
                            vt[:].rearrange("p (blk h d) -> p blk h d",
                                            blk=NTOK // 128, h=H)[
                                :, blk, half * 4:(half + 1) * 4, :
                            ],
                            ps[:].rearrange("p (h d) -> p h d", h=4),
                        )
                for h in range(H):
                    # ---- per-head q/k projection (SBUF only) ----
                    qh = bhpool.tile([128, NTOK], F16, name="qh", tag="qh")
                    kh = bhpool.tile([128, NTOK], F16, name="kh", tag="kh")
                    for wt_, outd, eng in (
                        (wk_t, kh, nc.scalar),
                        (wq_t, qh, nc.vector),
                    ):
                        for tt in range(TT):
                            ps = psA.tile([128, 512], F32, name="psp", tag="psa")
                            for kc in range(KC):
                                nc.tensor.matmul(
                                    ps[:],
                                    wt_[:, kc * D + h * 128: kc * D + (h + 1) * 128],
                                    xsl(kc, tt * 512, 512),
                                    start=(kc == 0),
                                    stop=(kc == KC - 1),
                                )
                            if eng is nc.scalar:
                                nc.scalar.activation(
                                    outd[:, tt * 512:(tt + 1) * 512], ps[:],
                                    AF.Copy)
                            else:
                                nc.vector.tensor_copy(
                                    outd[:, tt * 512:(tt + 1) * 512], ps[:])

                    # ---- banded attention for this head ----
                    for b0 in range(0, BC, 2):
                        dn = psB.tile([128, 512], F32, name="dn", tag="dn")
                        rr = bhpool.tile([128, 512], F16, name="rr", tag="rr")
                        avs = []
                        for bl in range(2):
                            b = b0 + bl
                            qb = qh[:, b * T:(b + 1) * T]
                            kb = kh[:, b * T:(b + 1) * T]
                            vb0 = vt[:, ((b * TC + 0) * H + h) * 128:
                                     ((b * TC + 0) * H + h) * 128 + 128]
                            vb1 = vt[:, ((b * TC + 1) * H + h) * 128:
                                     ((b * TC + 1) * H + h) * 128 + 128]
                            ps = psB.tile([128, 264], F32, name="psst", tag="psst")
                            nc.tensor.matmul(ps[:, 0:128], kb[:, 0:128],
                                             qb[:, 0:128], start=True, stop=True)
                            nc.tensor.matmul(ps[:, 256:259], kb[:, 0:128],
                                             qb[:, 128:131], start=True, stop=True)
                            nc.tensor.matmul(ps[:, 128:256], kb[:, 128:256],
                                             qb[:, 128:256], start=True, stop=True)
                            nc.tensor.matmul(ps[:, 259:262], kb[:, 128:256],
                                             qb[:, 125:128], start=True, stop=True)
                            pe = bhpool.tile([128, 264], F16, name="pe", tag="pe")
                            nc.scalar.activation(pe[:, 0:CW], ps[:, 0:CW],
                                                 AF.Exp, scale=SCALE)
                            pm = bhpool.tile([128, 264], F16, name="pm", tag="pm")
                            nc.vector.tensor_mul(pm[:, 0:CW], pe[:, 0:CW],
                                                 band_t[:, 0:CW])
                            dsl = dn[:, bl * 256:(bl + 1) * 256]
                            nc.tensor.matmul(dsl[:, 0:128], ones_t[:],
                                             pm[:, 0:128], start=True, stop=False)
                            nc.tensor.matmul(dsl[:, 125:128], ones_t[:, 0:128],
                                             pm[:, 259:262], start=False, stop=True,
                                             skip_group_check=True)
                            nc.tensor.matmul(dsl[:, 128:256], ones_t[:],
                                             pm[:, 128:256], start=True, stop=False)
                            nc.tensor.matmul(dsl[:, 128:131], ones_t[:, 0:128],
                                             pm[:, 256:259], start=False, stop=True,
                                             skip_group_check=True)
                            av = psB.tile([128, 256], F32, name="av", tag="av")
                            nc.tensor.matmul(av[:, 0:128], vb0,
                                             pm[:, 0:128], start=True, stop=False)
                            nc.tensor.matmul(av[:, 125:128], vb1,
                                             pm[:, 259:262], start=False, stop=True,
                                             skip_group_check=True)
                            nc.tensor.matmul(av[:, 128:256], vb1,
                                             pm[:, 128:256], start=True, stop=False)
                            nc.tensor.matmul(av[:, 128:131], vb0,
                                             pm[:, 256:259], start=False, stop=True,
                                             skip_group_check=True)
                            avs.append(av)
                        nc.vector.reciprocal(rr[:], dn[:])
                        for bl in range(2):
                            b = b0 + bl
                            nc.vector.tensor_mul(
                                ao_t[:, h * NTOK + b * T: h * NTOK + (b + 1) * T],
                                avs[bl][:], rr[:, bl * 256:(bl + 1) * 256],
                            )

                # ---- O projection + residual (in place) ----
                for half in range(2):
                    wo_t = wpool.tile([128, KC * 512], F16, name="wo_t",
                                      tag="wo", bufs=1)
                    nc.sync.dma_start(
                        wo_t[:].rearrange("p (c w) -> p c w", c=KC),
                        wo[l][:, half * 512:(half + 1) * 512]
                        .rearrange("(c p) w -> p c w", p=128),
                    )
                    for mcl in range(4):
                        mc = half * 4 + mcl
                        for tt in range(TT):
                            ps = psA.tile([128, 512], F32, name="pso", tag="psa")
                            for kc in range(KC):
                                nc.tensor.matmul(
                                    ps[:],
                                    wo_t[:, kc * 512 + mcl * 128: kc * 512 + (mcl + 1) * 128],
                                    ao_t[:, kc * NTOK + tt * 512: kc * NTOK + (tt + 1) * 512],
                                    start=(kc == 0),
                                    stop=(kc == KC - 1),
                                )
                            nc.vector.tensor_add(
                                xsl(mc, tt * 512, 512), ps[:], xsl(mc, tt * 512, 512)
                            )

        def gxb_phase(wpool, psA, psB):
            # gx_bwd -> DRAM, token-major via PE transposes (contiguous DMA)
            wb_t = wpool.tile([128, KC * G3], F16, name="wb_t", tag="wb",
                              bufs=1)
            nc.sync.dma_start(
                wb_t[:].rearrange("p (c g) -> p c g", c=KC),
                wihb[:, :].rearrange("(c p) g -> p c g", p=128),
            )
            for tt in range(TT):
                stg = wpool.tile([128, 4 * G3], F16, name="stg", tag="stg",
                                 bufs=2)
                for mc in range(GC):
                    ps = psA.tile([128, 512], F32, name="psg", tag="psa")
                    for kc in range(KC):
                        nc.tensor.matmul(
                            ps[:],
                            wb_t[:, kc * G3 + mc * 128: kc * G3 + (mc + 1) * 128],
                            xsl(kc, tt * 512, 512),
                            start=(kc == 0),
                            stop=(kc == KC - 1),
                        )
                    st = spool.tile([128, 512], F16, name="stg16", tag="st")
                    nc.scalar.activation(st[:], ps[:], AF.Copy)
                    for sub in range(4):
                        tp = psB.tile([128, 128], F16, name="tpd", tag="psst")
                        nc.tensor.transpose(
                            tp[:], st[:, sub * 128:(sub + 1) * 128], iden_t[:]
                        )
                        nc.vector.tensor_copy(
                            stg[:, sub * G3 + mc * 128: sub * G3 + (mc + 1) * 128],
                            tp[:],
                        )
                nc.sync.dma_start(
                    gxb_d[tt * 512:(tt + 1) * 512, :]
                    .rearrange("(sub p) g -> p sub g", p=128),
                    stg[:].rearrange("p (sub g) -> p sub g", sub=4),
                )

        def gru_phase(gxpool, recpool, psR):
            wf_t = gxpool.tile([128, KC * G3], F16, name="wf_t", tag="wf", bufs=1)
            nc.sync.dma_start(
                wf_t[:].rearrange("p (c g) -> p c g", c=KC),
                wihf[:, :].rearrange("(c p) g -> p c g", p=128),
            )
            h16prev = None
            for ck in range(NCHUNK):
                gxs = gxpool.tile([128, CH * 96], F16, name="gxs", tag="gxs")
                # fwd gx: compute directly into SBUF for this time chunk
                # gxs per-step layout: [xr0 xr1 xz0 xz1 xn0 xn1] (16 each) so
                # both direction chains run as single wide ops.
                for mc in range(GC):
                    ps = psR.tile([128, 256], F32, name="psf", tag="psf")
                    for kc in range(KC):
                        nc.tensor.matmul(
                            ps[:],
                            wf_t[:, kc * G3 + mc * 128: kc * G3 + (mc + 1) * 128],
                            x_t[:, kc * NTOK:(kc + 1) * NTOK]
                            .rearrange("p (b t) -> p b t", b=BC)[:, :, ck * CH:(ck + 1) * CH],
                            start=(kc == 0),
                            stop=(kc == KC - 1),
                        )
                    nc.vector.tensor_copy(
                        gxs[:, :]
                        .rearrange("p (j gp d c2 b) -> p j gp d c2 b",
                                   j=CH, gp=3, d=2, c2=2)[
                            :, :, mc // 2, 0, mc % 2, :
                        ].rearrange("p j b -> p b j"),
                        ps[:].rearrange("p (b j) -> p b j", b=BC),
                    )
                # bwd gx: indirect row gather in reverse_padded order + transpose
                for hf2 in range(2):
                    gb = gxpool.tile([128, G3], F16, name="gb", tag="gb", bufs=2)
                    nc.gpsimd.indirect_dma_start(
                        out=gb[:],
                        out_offset=None,
                        in_=gxb_d[:, :],
                        in_offset=bass.IndirectOffsetOnAxis(
                            ap=gxidx_t[:, ck * 2 + hf2: ck * 2 + hf2 + 1], axis=0
                        ),
                    )
                    for c in range(GC):
                        tp = psR.tile([128, 128], F16, name="tp", tag="tp")
                        nc.tensor.transpose(
                            tp[:], gb[:, c * 128:(c + 1) * 128], iden_t[:]
                        )
                        nc.vector.tensor_copy(
                            gxs[:, :]
                            .rearrange("p (j gp d c2 b) -> p j gp d c2 b",
                                       j=CH, gp=3, d=2, c2=2)[
                                :, :, c // 2, 1, c % 2, hf2 * 4:(hf2 + 1) * 4
                            ]
                            .rearrange("p j b -> p b j"),
                            tp[:].rearrange("p (b j) -> p b j", b=4),
                        )
                # y/h tile: [128, (j, dr, c, b)] fp16; the matmul moving
                # operand, the h for the gate blend, and the staged y are
                # all this one tile.
                h16t = recpool.tile([128, CH * 32], F16, name="h16t",
                                    tag="h16t", bufs=2)
                for jj in range(CH):
                    gsl = gxs[:, jj * 96:(jj + 1) * 96]
                    if jj == 0:
                        hs16 = hzero if h16prev is None else h16prev
                        hoff = 0 if h16prev is None else (CH - 1) * 32
                    else:
                        hs16, hoff = h16t, (jj - 1) * 32
                    hsl = hs16[:, hoff:hoff + 32]
                    ps_g = psR.tile([128, 96], F32, name="ps_g", tag="ps_g")
                    # inject 64*gx for r,z of both dirs; whh mms accumulate.
                    # ps_g cols: [r0 r1 z0 z1 n0 n1] (16 each)
                    nc.tensor.matmul(
                        ps_g[:, 0:64], iden64_t[:], gsl[:, 0:64],
                        start=True, stop=False,
                    )
                    # all r,z mms (both dirs) must precede any n-gate
                    # start=True: a start clears has_written for the WHOLE
                    # bank, killing accumulation for still-open groups.
                    for c_list in (range(4), range(4, GC)):
                        for dr in range(2):
                            for c in c_list:
                                col = (c // 2) * 32 + dr * 16 + (c % 2) * 8
                                for kc in range(HC):
                                    nc.tensor.matmul(
                                        ps_g[:, col:col + 8],
                                        whh_t[:, (dr * HC + kc) * G3 + c * 128:
                                              (dr * HC + kc) * G3 + (c + 1) * 128],
                                        hs16[:, hoff + dr * 16 + kc * 8:
                                             hoff + dr * 16 + (kc + 1) * 8],
                                        start=(c >= 4 and kc == 0),
                                        stop=(dr == 1 and c == 3 and kc == HC - 1)
                                        if c < 4 else (kc == HC - 1),
                                        skip_group_check=True,
                                    )
                    rz = recpool.tile([128, 64], F32, name="rz", tag="rz")
                    nc.scalar.activation(rz[:], ps_g[:, 0:64], AF.Sigmoid,
                                         scale=WHH_INV)
                    # off the critical chain: zc = 1 - z, zh = z * h
                    zc = recpool.tile([128, 32], F32, name="zc", tag="zc")
                    nc.gpsimd.tensor_sub(zc[:], ones_t[:, 0:32], rz[:, 32:64])
                    zh = recpool.tile([128, 32], F32, name="zh", tag="zh")
                    nc.gpsimd.tensor_mul(zh[:], rz[:, 32:64], hsl)
                    t1 = recpool.tile([128, 32], F32, name="t1", tag="t1")
                    nc.vector.tensor_mul(t1[:], rz[:, 0:32], ps_g[:, 64:96])
                    t2 = recpool.tile([128, 32], F32, name="t2", tag="t2")
                    nc.vector.scalar_tensor_tensor(
                        t2[:], t1[:], WHH_INV, gsl[:, 64:96],
                        op0=ALU.mult, op1=ALU.add,
                    )
                    n_t = recpool.tile([128, 32], F32, name="n_t", tag="n_t")
                    nc.scalar.activation(n_t[:], t2[:], AF.Tanh)
                    u_t = recpool.tile([128, 32], F32, name="u_t", tag="u_t")
                    nc.vector.tensor_mul(u_t[:], zc[:], n_t[:])
                    nc.vector.tensor_add(
                        h16t[:, jj * 32:(jj + 1) * 32], u_t[:], zh[:],
                    )
                # transpose to token-major and scatter into yout
                for dr in range(2):
                    for jh in range(2):
                        yrp = recpool.tile([128, 256], F16, name="yrp",
                                           tag="yrp", bufs=2)
                        for c in range(HC):
                            nc.vector.tensor_copy(
                                yrp[:, c * 128:(c + 1) * 128]
                                .rearrange("p (j b) -> p j b", j=16),
                                h16t[:, :]
                                .rearrange("p (j d c b) -> p j d c b",
                                           j=CH, d=2, c=HC)[
                                    :, jh * 16:(jh + 1) * 16, dr, c, :
                                ],
                            )
                        tp = psR.tile([128, 256], F16, name="tps", tag="tp")
                        for c in range(HC):
                            nc.tensor.transpose(
                                tp[:, c * 128:(c + 1) * 128],
                                yrp[:, c * 128:(c + 1) * 128],
                                iden_t[:],
                            )
                        yst = recpool.tile([128, 256], F16, name="yst",
                                           tag="yst", bufs=3)
                        nc.vector.tensor_copy(yst[:], tp[:])
                        col = ck * 4 + dr * 2 + jh
                        # sidx rows hold 2*(b*T+t)+dr: yout is [2*YR+2, GH]
                        # so that lands on token row (b*T+t), direction half dr.
                        nc.gpsimd.indirect_dma_start(
                            out=yout[:, :],
                            out_offset=bass.IndirectOffsetOnAxis(
                                ap=sidx_t[:, col:col + 1], axis=0
                            ),
                            in_=yst[:],
                            in_offset=None,
                        )
                h16prev = h16t

        for rep in range(repeat):
            if phases in ("all", "attn"):
                with (
                    tc.tile_pool(name="wt", bufs=1) as wpool,
                    tc.tile_pool(name="ao", bufs=1) as aopool,
                    tc.tile_pool(name="bh", bufs=2) as bhpool,
                    tc.tile_pool(name="psA", bufs=2, space="PSUM") as psA,
                    tc.tile_pool(name="psB", bufs=2, space="PSUM") as psB,
                ):
                    attn_phase(wpool, aopool, bhpool, psA, psB)
            if phases in ("all", "attn", "gxb"):
                with (
                    tc.tile_pool(name="wt2", bufs=1) as wpool2,
                    tc.tile_pool(name="psA2", bufs=2, space="PSUM") as psA2,
                    tc.tile_pool(name="psB2", bufs=2, space="PSUM") as psB2,
                ):
                    gxb_phase(wpool2, psA2, psB2)
            if phases in ("all", "gru"):
                with (
                    tc.tile_pool(name="gx", bufs=2) as gxpool,
                    tc.tile_pool(name="rec", bufs=3) as recpool,
                    tc.tile_pool(name="psR", bufs=2, space="PSUM") as psR,
                ):
                    gru_phase(gxpool, recpool, psR)

    nc.compile()
    return nc


_NC_CACHE = {}


def _get_nc(repeat: int = 1):
    if repeat not in _NC_CACHE:
        _NC_CACHE[repeat] = _build(repeat)
    return _NC_CACHE[repeat]


def _host_inputs(inputs, core):
    import ml_dtypes
    f8 = ml_dtypes.float8_e4m3

    bs = slice(core * BC, (core + 1) * BC)
    seg = np.asarray(inputs["seg_feats"][bs])
    seglen = np.asarray(inputs["seglen"][bs]).astype(np.int64)

    m = {
        "xT": np.ascontiguousarray(
            seg.transpose(2, 0, 1).reshape(D, NTOK)
        ).astype(np.float16)
    }
    for l in range(NL):
        for nm_in, nm_out in (("Wq", "WqT"), ("Wk", "WkT"), ("Wv", "WvT"),
                              ("Wo", "WoT")):
            m[f"{nm_out}{l}"] = np.ascontiguousarray(
                np.asarray(inputs[nm_in][l]).T).astype(np.float16)
    m["WihFT"] = np.ascontiguousarray(
        np.asarray(inputs["W_ih_f"]).T).astype(np.float16)
    m["WihBT"] = np.ascontiguousarray(
        np.asarray(inputs["W_ih_b"]).T).astype(np.float16)
    # biases are all zero in this model; the kernel skips them entirely
    for l in range(NL):
        for w in "qkvo":
            assert not np.any(np.asarray(inputs[f"b{w}"][l])), \
                "nonzero attention biases unsupported"
    for nm in ("b_ih_f", "b_ih_b", "b_hh_f", "b_hh_b"):
        assert not np.any(np.asarray(inputs[nm])), "nonzero GRU biases unsupported"
    m["WhhFT"] = np.ascontiguousarray(
        np.asarray(inputs["W_hh_f"]).T * WHH_SCALE).astype(f8)
    m["WhhBT"] = np.ascontiguousarray(
        np.asarray(inputs["W_hh_b"]).T * WHH_SCALE).astype(f8)

    # band mask: two 128x128 diagonal blocks + two 3-wide corner blocks
    band = np.zeros((128, 264), np.float32)
    p = np.arange(128)
    for c in range(2):
        band[:, c * 128:(c + 1) * 128] = (
            np.abs(p[:, None] - p[None, :]) <= ATTN_WIDTH
        )
    for j in range(3):
        for pp in range(125, 128):           # corner A: k=pp, q=128+j
            if abs(pp - 128 - j) <= ATTN_WIDTH:
                band[pp, 256 + j] = 1.0
        for pp in range(0, 3):               # corner B: k=128+pp, q=125+j
            if abs(128 + pp - 125 - j) <= ATTN_WIDTH:
                band[pp, 259 + j] = 1.0
    m["band"] = band.astype(np.float16)
    m["ones"] = np.ones((128, 128), np.float16)
    m["iden"] = np.eye(128, dtype=np.float16)
    m["iden64"] = (WHH_SCALE * np.eye(128)).astype(np.float16)

    gxidx = np.zeros((128, NCHUNK * 2), np.int32)
    for ck in range(NCHUNK):
        for hf2 in range(2):
            col = ck * 2 + hf2
            for bl in range(4):
                b = hf2 * 4 + bl
                L = int(seglen[b])
                for jl in range(CH):
                    j = ck * CH + jl
                    src_t = min(max(L - 1 - j, 0), T - 1)
                    gxidx[bl * CH + jl, col] = b * T + src_t
    m["gxidx"] = gxidx

    # scatter rows: partition p = (jl, b) of the transposed y block.
    # yout is [2*YR+2, GH]; row 2*(b*T+t)+dr is token (b,t), direction dr.
    sidx = np.full((128, NCHUNK * 4), 2 * YR, np.int32)
    for ck in range(NCHUNK):
        for dr in range(2):
            for jh in range(2):
                col = ck * 4 + dr * 2 + jh
                for jl in range(16):
                    j = ck * CH + jh * 16 + jl
                    for b in range(BC):
                        L = int(seglen[b])
                        if j < L:
                            t = j if dr == 0 else L - 1 - j
                            sidx[jl * 8 + b, col] = 2 * (b * T + t) + dr
    m["sidx"] = sidx
    return m


def core_output(yout_arr):
    return np.asarray(yout_arr)[0:2 * YR].reshape(BC, T, HID)


def kernel(**inputs) -> np.ndarray:
    repeat = int(os.environ.get("KERNEL_REPEAT", "1"))
    nc = _get_nc(repeat)
    in_maps = [_host_inputs(inputs, c) for c in range(NCORES)]
    res = run_bass_kernel_spmd(nc, in_maps, core_ids=list(range(NCORES)))
    out = np.stack([core_output(res.results[c]["yout"]) for c in range(NCORES)])
    return np.ascontiguousarray(
        out.reshape(B, T, HID), dtype=np.float32
    )


# revision 45
# speedup vs baseline: 4.9812x; 1.0246x over previous
"""Trainium2 Bass kernel for the CMIN video encoder (2x banded MHA + BiGRU).

Self-contained: builds one SPMD Bass program, shards batch across the
8 NeuronCores (8 batches each), runs via run_bass_kernel_spmd, and
reassembles the full [64, 256, 512] output on the host.

Layout: activations feature-major f16 ([feature, token]); projections are
lhsT=weightT matmuls. Attention is fused per head entirely in SBUF (q/k/v
never touch DRAM); the band never leaves the two 128x128 diagonal score
blocks plus two 3-wide corners. gx_bwd is staged to DRAM token-major via
PE transposes + contiguous DMA (no scatter). The BiGRU runs both direction
chains interleaved; W_hh in fp8e4 (x64 prescale); gx is injected into the
gate PSUM group by a 64*I matmul so the sigmoid reads PSUM directly with
the free affine 1/64 scale. Every 32 steps the y-tile is PE-transposed to
token-major and indirect-DMA-scattered into the output tensor - sequence
reversal, placement and tail masking all encoded in host-built row-index
tables, keeping the program SPMD-identical.
"""

import os
import numpy as np
import concourse.bass as bass
import concourse.bacc as bacc
import concourse.tile as tile
import concourse.mybir as mybir
from concourse.bass_utils import run_bass_kernel_spmd

B, T, D = 64, 256, 1024
H, DK = 8, D // 8
HID = 512
GH = HID >> 1          # 256
G3 = 3 * GH            # 768
ATTN_WIDTH = 3
NL = 2
NCORES = 8
BC = B // NCORES       # 8 batches per core
NTOK = BC * T          # 2048 token columns per core
SCALE = 1.0 / float(np.sqrt(DK))

F32 = mybir.dt.float32
F16 = mybir.dt.float16
F8 = mybir.dt.float8e4
I32 = mybir.dt.int32
AF = mybir.ActivationFunctionType
ALU = mybir.AluOpType

KC = D // 128          # 8 contraction chunks for D
GC = G3 // 128         # 6 gate chunks
HC = GH // 128         # 2 hidden chunks
TT = NTOK // 512       # 4 token tiles of 512
TC = T // 128          # 2 chunks of the T axis
CH = 32                # recurrence steps per gx stream chunk
NCHUNK = T // CH

YR = BC * T            # valid output tokens; row 2*YR is the trash row
WHH_SCALE = 64.0
WHH_INV = 1.0 / WHH_SCALE
WSC = 32.0             # fp8 weight prescale (keeps N(0,0.02) weights normal)
WSC_INV = 1.0 / WSC
CW = 262               # valid score columns: 2x128 diag + 2x3 corners
DR = None              # set at build: mybir.MatmulPerfMode.DoubleRow


def _build(repeat: int = 1, phases: str = "all"):
    nc = bacc.Bacc("TRN2", num_devices=NCORES)

    DRM = mybir.MatmulPerfMode.DoubleRow

    xT = nc.dram_tensor("xT", [D, NTOK], F16, kind="ExternalInput")
    wq, wk, wv, wo = [], [], [], []
    for l in range(NL):
        wq.append(nc.dram_tensor(f"WqT{l}", [D, D], F8, kind="ExternalInput"))
        wk.append(nc.dram_tensor(f"WkT{l}", [D, D], F8, kind="ExternalInput"))
        wv.append(nc.dram_tensor(f"WvT{l}", [D, D], F16, kind="ExternalInput"))
        wo.append(nc.dram_tensor(f"WoT{l}", [D, D], F16, kind="ExternalInput"))
    wihf = nc.dram_tensor("WihFT", [D, G3], F16, kind="ExternalInput")
    wihb = nc.dram_tensor("WihBT", [D, G3], F16, kind="ExternalInput")
    whhf = nc.dram_tensor("WhhFT", [GH, G3], F8, kind="ExternalInput")
    whhb = nc.dram_tensor("WhhBT", [GH, G3], F8, kind="ExternalInput")
    band_d = nc.dram_tensor("band", [128, 264], F16, kind="ExternalInput")
    ones_d = nc.dram_tensor("ones", [128, 128], F16, kind="ExternalInput")
    iden_d = nc.dram_tensor("iden", [128, 128], F16, kind="ExternalInput")
    iden64_d = nc.dram_tensor("iden64", [128, 128], F16, kind="ExternalInput")
    gxidx_d = nc.dram_tensor("gxidx", [128, NCHUNK * 2], I32, kind="ExternalInput")
    sidx_d = nc.dram_tensor("sidx", [128, NCHUNK * 4], I32, kind="ExternalInput")
    yout = nc.dram_tensor("yout", [2 * YR + 2, GH], F16, kind="ExternalOutput")

    with (
        nc.allow_low_precision(reason="f16/fp8 staging is deliberate"),
        tile.TileContext(nc) as tc,
        tc.tile_pool(name="dram", bufs=1, space="DRAM") as dpool,
        tc.tile_pool(name="const", bufs=1) as cpool,
        tc.tile_pool(name="xs", bufs=1) as xpool,
        tc.tile_pool(name="stage", bufs=6) as spool,
    ):
        gxb_d = dpool.tile([NTOK, G3], F16, name="gxb_d")

        # ---- constants ---------------------------------------------------
        band_t = cpool.tile([128, 264], F16, name="band_t")
        nc.sync.dma_start(band_t[:], band_d[:])
        ones_t = cpool.tile([128, 128], F16, name="ones_t")
        nc.sync.dma_start(ones_t[:], ones_d[:])
        iden_t = cpool.tile([128, 128], F16, name="iden_t")
        nc.sync.dma_start(iden_t[:], iden_d[:])
        iden64_t = cpool.tile([128, 128], F16, name="iden64_t")
        nc.sync.dma_start(iden64_t[:], iden64_d[:])
        gxidx_t = cpool.tile([128, NCHUNK * 2], I32, name="gxidx_t")
        nc.sync.dma_start(gxidx_t[:], gxidx_d[:])
        sidx_t = cpool.tile([128, NCHUNK * 4], I32, name="sidx_t")
        nc.sync.dma_start(sidx_t[:], sidx_d[:])
        whh_t = cpool.tile([128, 2 * HC * G3], F8, name="whh_t")
        for dr, wd in enumerate((whhf, whhb)):
            nc.sync.dma_start(
                whh_t[:, dr * HC * G3:(dr + 1) * HC * G3]
                .rearrange("p (c g) -> p c g", c=HC),
                wd[:, :].rearrange("(c p) g -> p c g", p=128),
            )
        hzero = cpool.tile([128, 2 * HC * BC], F16, name="hzero")
        nc.vector.memset(hzero[:], 0.0)

        # ---- x resident (feature-major, f16 master + fp8 matmul copy) ----
        x_t = xpool.tile([128, KC * NTOK], F16, name="x_t")
        nc.sync.dma_start(
            x_t[:].rearrange("p (c n) -> p c n", c=KC),
            xT[:, :].rearrange("(c p) n -> p c n", p=128),
        )
        x8 = xpool.tile([128, KC * NTOK], F8, name="x8")
        for kc in range(KC):
            nc.gpsimd.tensor_copy(
                x8[:, kc * NTOK:(kc + 1) * NTOK],
                x_t[:, kc * NTOK:(kc + 1) * NTOK],
            )

        def xsl(kc, c0=0, n=NTOK):
            return x_t[:, kc * NTOK + c0: kc * NTOK + c0 + n]

        def x8v():
            return x8[:].rearrange("p (c n) -> p c n", c=KC)

        def attn_phase(wpool, aopool, bhpool, psA, psB):
            ao_t = aopool.tile([128, H * NTOK], F16, name="ao_t")
            for l in range(NL):
                # full-weight loads for q/k/v (sliced per head below).
                # fp8, host-prescaled x32: q/k/v/ao all carry the scale;
                # exp and the O-residual divide it back out.
                wq_t = wpool.tile([128, KC * D], F8, name="wq_t", tag="wq",
                                  bufs=1)
                wk_t = wpool.tile([128, KC * D], F8, name="wk_t", tag="wk",
                                  bufs=1)
                wv_t = wpool.tile([128, KC * D], F16, name="wv_t", tag="wv",
                                  bufs=1)
                for wt_, wd_ in ((wq_t, wq[l]), (wk_t, wk[l]), (wv_t, wv[l])):
                    nc.sync.dma_start(
                        wt_[:].rearrange("p (c d) -> p c d", c=KC),
                        wd_[:, :].rearrange("(c p) d -> p c d", p=128),
                    )
                # ---- V projection, token-major (the av matmuls contract
                # over k-tokens on partitions): vt[tok, (blk, h, dk)] ----
                vt = aopool.tile([128, (NTOK // 128) * H * 128], F16,
                                 name="vt_t", tag="vt")
                for half in range(2):
                    for blk in range(NTOK // 128):
                        ps = psA.tile([128, 512], F32, name="psv", tag="psa")
                        for kc in range(KC):
                            nc.tensor.matmul(
                                ps[:],
                                xsl(kc, blk * 128, 128),
                                wv_t[:, kc * D + half * 512:
                                     kc * D + half * 512 + 512],
                                start=(kc == 0),
                                stop=(kc == KC - 1),
                            )
                        nc.vector.tensor_copy(
                            vt[:].rearrange("p (blk h d) -> p blk h d",
                                            blk=NTOK // 128, h=H)[
                                :, blk, half * 4:(half + 1) * 4, :
                            ],
                            ps[:].rearrange("p (h d) -> p h d", h=4),
                        )
                for h in range(H):
                    # ---- per-head q/k projection (SBUF only) ----
                    qh = bhpool.tile([128, NTOK], F16, name="qh", tag="qh")
                    kh = bhpool.tile([128, NTOK], F16, name="kh", tag="kh")
                    for wt_, outd, eng in (
                        (wk_t, kh, nc.scalar),
                        (wq_t, qh, nc.vector),
                    ):
                        for tt in range(TT):
                            ps = psA.tile([128, 512], F32, name="psp", tag="psa")
                            for k2 in range(KC // 2):
                                nc.tensor.matmul(
                                    ps[:],
                                    wt_[:].rearrange("p (c d) -> p c d", c=KC)[
                                        :, 2 * k2:2 * k2 + 2,
                                        h * 128:(h + 1) * 128],
                                    x8v()[:, 2 * k2:2 * k2 + 2,
                                          tt * 512:(tt + 1) * 512],
                                    start=(k2 == 0),
                                    stop=(k2 == KC // 2 - 1),
                                    perf_mode=DRM,
                                )
                            if eng is nc.scalar:
                                nc.scalar.activation(
                                    outd[:, tt * 512:(tt + 1) * 512], ps[:],
                                    AF.Copy)
                            else:
                                nc.vector.tensor_copy(
                                    outd[:, tt * 512:(tt + 1) * 512], ps[:])

                    # ---- banded attention for this head ----
                    for b0 in range(0, BC, 2):
                        dn = psB.tile([128, 512], F32, name="dn", tag="dn")
                        rr = bhpool.tile([128, 512], F16, name="rr", tag="rr")
                        avs = []
                        for bl in range(2):
                            b = b0 + bl
                            qb = qh[:, b * T:(b + 1) * T]
                            kb = kh[:, b * T:(b + 1) * T]
                            vb0 = vt[:, ((b * TC + 0) * H + h) * 128:
                                     ((b * TC + 0) * H + h) * 128 + 128]
                            vb1 = vt[:, ((b * TC + 1) * H + h) * 128:
                                     ((b * TC + 1) * H + h) * 128 + 128]
                            ps = psB.tile([128, 264], F32, name="psst", tag="psst")
                            nc.tensor.matmul(ps[:, 0:128], kb[:, 0:128],
                                             qb[:, 0:128], start=True, stop=True)
                            nc.tensor.matmul(ps[:, 256:259], kb[:, 0:128],
                                             qb[:, 128:131], start=True, stop=True)
                            nc.tensor.matmul(ps[:, 128:256], kb[:, 128:256],
                                             qb[:, 128:256], start=True, stop=True)
                            nc.tensor.matmul(ps[:, 259:262], kb[:, 128:256],
                                             qb[:, 125:128], start=True, stop=True)
                            pe = bhpool.tile([128, 264], F16, name="pe", tag="pe")
                            nc.scalar.activation(pe[:, 0:CW], ps[:, 0:CW],
                                                 AF.Exp,
                                                 scale=SCALE / (WSC * WSC))
                            pm = bhpool.tile([128, 264], F16, name="pm", tag="pm")
                            nc.vector.tensor_mul(pm[:, 0:CW], pe[:, 0:CW],
                                                 band_t[:, 0:CW])
                            dsl = dn[:, bl * 256:(bl + 1) * 256]
                            nc.tensor.matmul(dsl[:, 0:128], ones_t[:],
                                             pm[:, 0:128], start=True, stop=False)
                            nc.tensor.matmul(dsl[:, 125:128], ones_t[:, 0:128],
                                             pm[:, 259:262], start=False, stop=True,
                                             skip_group_check=True)
                            nc.tensor.matmul(dsl[:, 128:256], ones_t[:],
                                             pm[:, 128:256], start=True, stop=False)
                            nc.tensor.matmul(dsl[:, 128:131], ones_t[:, 0:128],
                                             pm[:, 256:259], start=False, stop=True,
                                             skip_group_check=True)
                            av = psB.tile([128, 256], F32, name="av", tag="av")
                            nc.tensor.matmul(av[:, 0:128], vb0,
                                             pm[:, 0:128], start=True, stop=False)
                            nc.tensor.matmul(av[:, 125:128], vb1,
                                             pm[:, 259:262], start=False, stop=True,
                                             skip_group_check=True)
                            nc.tensor.matmul(av[:, 128:256], vb1,
                                             pm[:, 128:256], start=True, stop=False)
                            nc.tensor.matmul(av[:, 128:131], vb0,
                                             pm[:, 256:259], start=False, stop=True,
                                             skip_group_check=True)
                            avs.append(av)
                        nc.vector.reciprocal(rr[:], dn[:])
                        for bl in range(2):
                            b = b0 + bl
                            nc.vector.tensor_mul(
                                ao_t[:, h * NTOK + b * T: h * NTOK + (b + 1) * T],
                                avs[bl][:], rr[:, bl * 256:(bl + 1) * 256],
                            )

                # ---- O projection + residual (in place) ----
                for half in range(2):
                    wo_t = wpool.tile([128, KC * 512], F16, name="wo_t",
                                      tag="wo", bufs=1)
                    nc.sync.dma_start(
                        wo_t[:].rearrange("p (c w) -> p c w", c=KC),
                        wo[l][:, half * 512:(half + 1) * 512]
                        .rearrange("(c p) w -> p c w", p=128),
                    )
                    for mcl in range(4):
                        mc = half * 4 + mcl
                        for tt in range(TT):
                            ps = psA.tile([128, 512], F32, name="pso", tag="psa")
                            for kc in range(KC):
                                nc.tensor.matmul(
                                    ps[:],
                                    wo_t[:, kc * 512 + mcl * 128:
                                         kc * 512 + (mcl + 1) * 128],
                                    ao_t[:, kc * NTOK + tt * 512:
                                         kc * NTOK + (tt + 1) * 512],
                                    start=(kc == 0),
                                    stop=(kc == KC - 1),
                                )
                            nc.vector.tensor_add(
                                xsl(mc, tt * 512, 512), ps[:],
                                xsl(mc, tt * 512, 512),
                            )
                            nc.gpsimd.tensor_copy(
                                x8[:, mc * NTOK + tt * 512:
                                   mc * NTOK + (tt + 1) * 512],
                                xsl(mc, tt * 512, 512),
                            )

        def gxb_phase(wpool, psA, psB):
            # gx_bwd -> DRAM, token-major via PE transposes (contiguous DMA)
            wb_t = wpool.tile([128, KC * G3], F16, name="wb_t", tag="wb",
                              bufs=1)
            nc.sync.dma_start(
                wb_t[:].rearrange("p (c g) -> p c g", c=KC),
                wihb[:, :].rearrange("(c p) g -> p c g", p=128),
            )
            for tt in range(TT):
                stg = wpool.tile([128, 4 * G3], F16, name="stg", tag="stg",
                                 bufs=2)
                for mc in range(GC):
                    ps = psA.tile([128, 512], F32, name="psg", tag="psa")
                    for kc in range(KC):
                        nc.tensor.matmul(
                            ps[:],
                            wb_t[:, kc * G3 + mc * 128: kc * G3 + (mc + 1) * 128],
                            xsl(kc, tt * 512, 512),
                            start=(kc == 0),
                            stop=(kc == KC - 1),
                        )
                    st = spool.tile([128, 512], F16, name="stg16", tag="st")
                    nc.scalar.activation(st[:], ps[:], AF.Copy)
                    for sub in range(4):
                        tp = psB.tile([128, 128], F16, name="tpd", tag="psst")
                        nc.tensor.transpose(
                            tp[:], st[:, sub * 128:(sub + 1) * 128], iden_t[:]
                        )
                        nc.vector.tensor_copy(
                            stg[:, sub * G3 + mc * 128: sub * G3 + (mc + 1) * 128],
                            tp[:],
                        )
                nc.sync.dma_start(
                    gxb_d[tt * 512:(tt + 1) * 512, :]
                    .rearrange("(sub p) g -> p sub g", p=128),
                    stg[:].rearrange("p (sub g) -> p sub g", sub=4),
                )

        def gru_phase(gxpool, recpool, psR):
            wf_t = gxpool.tile([128, KC * G3], F16, name="wf_t", tag="wf", bufs=1)
            nc.sync.dma_start(
                wf_t[:].rearrange("p (c g) -> p c g", c=KC),
                wihf[:, :].rearrange("(c p) g -> p c g", p=128),
            )
            h16prev = None
            for ck in range(NCHUNK):
                gxs = gxpool.tile([128, CH * 96], F16, name="gxs", tag="gxs")
                # fwd gx: compute directly into SBUF for this time chunk
                # gxs per-step layout: [xr0 xr1 xz0 xz1 xn0 xn1] (16 each) so
                # both direction chains run as single wide ops.
                for mc in range(GC):
                    ps = psR.tile([128, 256], F32, name="psf", tag="psf")
                    for kc in range(KC):
                        nc.tensor.matmul(
                            ps[:],
                            wf_t[:, kc * G3 + mc * 128: kc * G3 + (mc + 1) * 128],
                            x_t[:, kc * NTOK:(kc + 1) * NTOK]
                            .rearrange("p (b t) -> p b t", b=BC)[
                                :, :, ck * CH:(ck + 1) * CH],
                            start=(kc == 0),
                            stop=(kc == KC - 1),
                        )
                    nc.vector.tensor_copy(
                        gxs[:, :]
                        .rearrange("p (j gp d c2 b) -> p j gp d c2 b",
                                   j=CH, gp=3, d=2, c2=2)[
                            :, :, mc // 2, 0, mc % 2, :
                        ].rearrange("p j b -> p b j"),
                        ps[:].rearrange("p (b j) -> p b j", b=BC),
                    )
                # bwd gx: indirect row gather in reverse_padded order + transpose
                for hf2 in range(2):
                    gb = gxpool.tile([128, G3], F16, name="gb", tag="gb", bufs=2)
                    nc.gpsimd.indirect_dma_start(
                        out=gb[:],
                        out_offset=None,
                        in_=gxb_d[:, :],
                        in_offset=bass.IndirectOffsetOnAxis(
                            ap=gxidx_t[:, ck * 2 + hf2: ck * 2 + hf2 + 1], axis=0
                        ),
                    )
                    for c in range(GC):
                        tp = psR.tile([128, 128], F16, name="tp", tag="tp")
                        nc.tensor.transpose(
                            tp[:], gb[:, c * 128:(c + 1) * 128], iden_t[:]
                        )
                        nc.vector.tensor_copy(
                            gxs[:, :]
                            .rearrange("p (j gp d c2 b) -> p j gp d c2 b",
                                       j=CH, gp=3, d=2, c2=2)[
                                :, :, c // 2, 1, c % 2, hf2 * 4:(hf2 + 1) * 4
                            ]
                            .rearrange("p j b -> p b j"),
                            tp[:].rearrange("p (b j) -> p b j", b=4),
                        )
                # y/h tile: [128, (j, dr, c, b)] fp16; the matmul moving
                # operand, the h for the gate blend, and the staged y are
                # all this one tile.
                h16t = recpool.tile([128, CH * 32], F16, name="h16t",
                                    tag="h16t", bufs=2)
                for jj in range(CH):
                    gsl = gxs[:, jj * 96:(jj + 1) * 96]
                    if jj == 0:
                        hs16 = hzero if h16prev is None else h16prev
                        hoff = 0 if h16prev is None else (CH - 1) * 32
                    else:
                        hs16, hoff = h16t, (jj - 1) * 32
                    hsl = hs16[:, hoff:hoff + 32]
                    ps_g = psR.tile([128, 96], F32, name="ps_g", tag="ps_g")
                    # inject 64*gx for r,z of both dirs; whh mms accumulate.
                    # ps_g cols: [r0 r1 z0 z1 n0 n1] (16 each)
                    nc.tensor.matmul(
                        ps_g[:, 0:64], iden64_t[:], gsl[:, 0:64],
                        start=True, stop=False,
                    )
                    # all r,z mms (both dirs) must precede any n-gate
                    # start=True: a start clears has_written for the WHOLE
                    # bank, killing accumulation for still-open groups.
                    for c_list in (range(4), range(4, GC)):
                        for dr in range(2):
                            for c in c_list:
                                col = (c // 2) * 32 + dr * 16 + (c % 2) * 8
                                for kc in range(HC):
                                    nc.tensor.matmul(
                                        ps_g[:, col:col + 8],
                                        whh_t[:, (dr * HC + kc) * G3 + c * 128:
                                              (dr * HC + kc) * G3 + (c + 1) * 128],
                                        hs16[:, hoff + dr * 16 + kc * 8:
                                             hoff + dr * 16 + (kc + 1) * 8],
                                        start=(c >= 4 and kc == 0),
                                        stop=(dr == 1 and c == 3 and kc == HC - 1)
                                        if c < 4 else (kc == HC - 1),
                                        skip_group_check=True,
                                    )
                    rz = recpool.tile([128, 64], F32, name="rz", tag="rz")
                    nc.scalar.activation(rz[:], ps_g[:, 0:64], AF.Sigmoid,
                                         scale=WHH_INV)
                    # off the critical chain: zc = 1 - z, zh = z * h
                    zc = recpool.tile([128, 32], F32, name="zc", tag="zc")
                    nc.gpsimd.tensor_sub(zc[:], ones_t[:, 0:32], rz[:, 32:64])
                    zh = recpool.tile([128, 32], F32, name="zh", tag="zh")
                    nc.gpsimd.tensor_mul(zh[:], rz[:, 32:64], hsl)
                    t1 = recpool.tile([128, 32], F32, name="t1", tag="t1")
                    nc.vector.tensor_mul(t1[:], rz[:, 0:32], ps_g[:, 64:96])
                    t2 = recpool.tile([128, 32], F32, name="t2", tag="t2")
                    nc.vector.scalar_tensor_tensor(
                        t2[:], t1[:], WHH_INV, gsl[:, 64:96],
                        op0=ALU.mult, op1=ALU.add,
                    )
                    n_t = recpool.tile([128, 32], F32, name="n_t", tag="n_t")
                    nc.scalar.activation(n_t[:], t2[:], AF.Tanh)
                    u_t = recpool.tile([128, 32], F32, name="u_t", tag="u_t")
                    nc.vector.tensor_mul(u_t[:], zc[:], n_t[:])
                    nc.vector.tensor_add(
                        h16t[:, jj * 32:(jj + 1) * 32], u_t[:], zh[:],
                    )
                # transpose to token-major and scatter into yout
                for dr in range(2):
                    for jh in range(2):
                        yrp = recpool.tile([128, 256], F16, name="yrp",
                                           tag="yrp", bufs=2)
                        for c in range(HC):
                            nc.vector.tensor_copy(
                                yrp[:, c * 128:(c + 1) * 128]
                                .rearrange("p (j b) -> p j b", j=16),
                                h16t[:, :]
                                .rearrange("p (j d c b) -> p j d c b",
                                           j=CH, d=2, c=HC)[
                                    :, jh * 16:(jh + 1) * 16, dr, c, :
                                ],
                            )
                        tp = psR.tile([128, 256], F16, name="tps", tag="tp")
                        for c in range(HC):
                            nc.tensor.transpose(
                                tp[:, c * 128:(c + 1) * 128],
                                yrp[:, c * 128:(c + 1) * 128],
                                iden_t[:],
                            )
                        yst = recpool.tile([128, 256], F16, name="yst",
                                           tag="yst", bufs=3)
                        nc.vector.tensor_copy(yst[:], tp[:])
                        col = ck * 4 + dr * 2 + jh
                        # sidx rows hold 2*(b*T+t)+dr: yout is [2*YR+2, GH]
                        # so that lands on token row (b*T+t), direction half dr.
                        nc.gpsimd.indirect_dma_start(
                            out=yout[:, :],
                            out_offset=bass.IndirectOffsetOnAxis(
                                ap=sidx_t[:, col:col + 1], axis=0
                            ),
                            in_=yst[:],
                            in_offset=None,
                        )
                h16prev = h16t

        for rep in range(repeat):
            if phases in ("all", "attn"):
                with (
                    tc.tile_pool(name="wt", bufs=1) as wpool,
                    tc.tile_pool(name="ao", bufs=1) as aopool,
                    tc.tile_pool(name="bh", bufs=2) as bhpool,
                    tc.tile_pool(name="psA", bufs=2, space="PSUM") as psA,
                    tc.tile_pool(name="psB", bufs=2, space="PSUM") as psB,
                ):
                    attn_phase(wpool, aopool, bhpool, psA, psB)
            if phases in ("all", "attn", "gxb"):
                with (
                    tc.tile_pool(name="wt2", bufs=1) as wpool2,
                    tc.tile_pool(name="psA2", bufs=2, space="PSUM") as psA2,
                    tc.tile_pool(name="psB2", bufs=2, space="PSUM") as psB2,
                ):
                    gxb_phase(wpool2, psA2, psB2)
            if phases in ("all", "gru"):
                with (
                    tc.tile_pool(name="gx", bufs=2) as gxpool,
                    tc.tile_pool(name="rec", bufs=3) as recpool,
                    tc.tile_pool(name="psR", bufs=2, space="PSUM") as psR,
                ):
                    gru_phase(gxpool, recpool, psR)

    nc.compile()
    return nc


_NC_CACHE = {}


def _get_nc(repeat: int = 1):
    if repeat not in _NC_CACHE:
        _NC_CACHE[repeat] = _build(repeat)
    return _NC_CACHE[repeat]


def _host_inputs(inputs, core):
    import ml_dtypes
    f8 = ml_dtypes.float8_e4m3

    bs = slice(core * BC, (core + 1) * BC)
    seg = np.asarray(inputs["seg_feats"][bs])
    seglen = np.asarray(inputs["seglen"][bs]).astype(np.int64)

    m = {
        "xT": np.ascontiguousarray(
            seg.transpose(2, 0, 1).reshape(D, NTOK)
        ).astype(np.float16)
    }
    for l in range(NL):
        for nm_in, nm_out in (("Wq", "WqT"), ("Wk", "WkT")):
            m[f"{nm_out}{l}"] = np.ascontiguousarray(
                np.asarray(inputs[nm_in][l]).T * WSC).astype(f8)
        for nm_in, nm_out in (("Wv", "WvT"), ("Wo", "WoT")):
            m[f"{nm_out}{l}"] = np.ascontiguousarray(
                np.asarray(inputs[nm_in][l]).T).astype(np.float16)
    m["WihFT"] = np.ascontiguousarray(
        np.asarray(inputs["W_ih_f"]).T).astype(np.float16)
    m["WihBT"] = np.ascontiguousarray(
        np.asarray(inputs["W_ih_b"]).T).astype(np.float16)
    # biases are all zero in this model; the kernel skips them entirely
    for l in range(NL):
        for w in "qkvo":
            assert not np.any(np.asarray(inputs[f"b{w}"][l])), \
                "nonzero attention biases unsupported"
    for nm in ("b_ih_f", "b_ih_b", "b_hh_f", "b_hh_b"):
        assert not np.any(np.asarray(inputs[nm])), "nonzero GRU biases unsupported"
    m["WhhFT"] = np.ascontiguousarray(
        np.asarray(inputs["W_hh_f"]).T * WHH_SCALE).astype(f8)
    m["WhhBT"] = np.ascontiguousarray(
        np.asarray(inputs["W_hh_b"]).T * WHH_SCALE).astype(f8)

    # band mask: two 128x128 diagonal blocks + two 3-wide corner blocks
    band = np.zeros((128, 264), np.float32)
    p = np.arange(128)
    for c in range(2):
        band[:, c * 128:(c + 1) * 128] = (
            np.abs(p[:, None] - p[None, :]) <= ATTN_WIDTH
        )
    for j in range(3):
        for pp in range(125, 128):           # corner A: k=pp, q=128+j
            if abs(pp - 128 - j) <= ATTN_WIDTH:
                band[pp, 256 + j] = 1.0
        for pp in range(0, 3):               # corner B: k=128+pp, q=125+j
            if abs(128 + pp - 125 - j) <= ATTN_WIDTH:
                band[pp, 259 + j] = 1.0
    m["band"] = band.astype(np.float16)
    m["ones"] = np.ones((128, 128), np.float16)
    m["iden"] = np.eye(128, dtype=np.float16)
    m["iden64"] = (WHH_SCALE * np.eye(128)).astype(np.float16)

    gxidx = np.zeros((128, NCHUNK * 2), np.int32)
    for ck in range(NCHUNK):
        for hf2 in range(2):
            col = ck * 2 + hf2
            for bl in range(4):
                b = hf2 * 4 + bl
                L = int(seglen[b])
                for jl in range(CH):
                    j = ck * CH + jl
                    src_t = min(max(L - 1 - j, 0), T - 1)
                    gxidx[bl * CH + jl, col] = b * T + src_t
    m["gxidx"] = gxidx

    # scatter rows: partition p = (jl, b) of the transposed y block.
    # yout is [2*YR+2, GH]; row 2*(b*T+t)+dr is token (b,t), direction dr.
    sidx = np.full((128, NCHUNK * 4), 2 * YR, np.int32)
    for ck in range(NCHUNK):
        for dr in range(2):
            for jh in range(2):
                col = ck * 4 + dr * 2 + jh
                for jl in range(16):
                    j = ck * CH + jh * 16 + jl
                    for b in range(BC):
                        L = int(seglen[b])
                        if j < L:
                            t = j if dr == 0 else L - 1 - j
                            sidx[jl * 8 + b, col] = 2 * (b * T + t) + dr
    m["sidx"] = sidx
    return m


def core_output(yout_arr):
    return np.asarray(yout_arr)[0:2 * YR].reshape(BC, T, HID)


def kernel(**inputs) -> np.ndarray:
    repeat = int(os.environ.get("KERNEL_REPEAT", "1"))
    nc = _get_nc(repeat)
    in_maps = [_host_inputs(inputs, c) for c in range(NCORES)]
    res = run_bass_kernel_spmd(nc, in_maps, core_ids=list(range(NCORES)))
    out = np.stack([core_output(res.results[c]["yout"]) for c in range(NCORES)])
    return np.ascontiguousarray(
        out.reshape(B, T, HID), dtype=np.float32
    )
